# revision 81
# baseline (speedup 1.0000x reference)
"""Trainium2 Bass kernel for nn_CMKConMambaBlock (ConMamba block).

Sharding: 8 NeuronCores = 4 batch x 2 d_inner-halves. Single SPMD program;
per-core differences are injected purely through host-prepared inputs
(weights reordered so each core's d-half is local slice 0; the MKGU
N-region is routed by a pairwise ReduceScatter). The Mamba selective scan
runs as 128 per-d tensor_tensor_scan recurrences in an
[s=128 partitions, t=1024 free] layout. Per d, the dt row is broadcast on
the PE via a per-d one-hot selector matmul into PSUM, Act exponentiates it
with the per-state A column, the dtx row is broadcast by a stride-0 DRAM
DMA on the otherwise-idle SP queue, and the b/scan/g elementwise chain is
split across DVE and Pool (scan itself is DVE-only on HW; Pool never
touches PSUM). The y reduction over states stays on PE via the sliding
one-hot. MKGU weights stream in on SP slack chunked inside the scan loop;
conv biases ride the PE accumulation as rank-1 ones-row taps; the
ReduceScatter payload is bf16. Elementwise work is spread across
DVE/Pool/Act per the legacy cost model rates (DVE 2x for packed bf16,
Act/Pool dtype-agnostic).

kernel(**inputs) takes the full unsharded inputs (as produced by
setup_inputs()) and returns the full (4, 1024, 128) float32 output.
"""
import sys
for _p in ("/opt/trn_rl_repo", "/root/.axon_site/_ro/trn_rl_repo"):
    if _p not in sys.path:
        sys.path.append(_p)


import contextlib
import numpy as np
import ml_dtypes

import concourse.bass as bass
import concourse.bacc as bacc
from concourse.bass_utils import run_bass_kernel_spmd
import concourse.tile as tile
from concourse import mybir

F32 = mybir.dt.float32
F32R = mybir.dt.float32r
BF16 = mybir.dt.bfloat16
AX = mybir.AluOpType
AF = mybir.ActivationFunctionType

B, N, C = 4, 1024, 128
DI, S, R, DC = 256, 128, 8, 4
EPS = 1e-5
NH = 512          # N half
EXTL = 32         # halo for MKGU region
EXT = NH + 2 * EXTL   # 576
MCW = NH + 32     # mc width needed for dw conv: [n0-16, n0+528)
N_CORES = 8
BF = ml_dtypes.bfloat16


# --------------------------------------------------------------------------
# Host-side input prep (numpy only; layout + weight reorg).
# --------------------------------------------------------------------------
def prep_core_inputs(d, b, q):
    """d: dict of full inputs (numpy). Returns in_map for core 2*b+q."""
    f32 = np.float32

    def bf(x):
        return np.ascontiguousarray(np.asarray(x, f32).astype(BF))

    def cc(x):
        return np.ascontiguousarray(np.asarray(x, f32))

    m = {}
    x = np.asarray(d['x'], f32)
    m['xT'] = cc(x[b].T)                                     # [C,N]
    m['xr'] = bf(x[b].reshape(C, N))                         # raw reshape
    m['eye'] = np.eye(128, dtype=f32)
    m['eye_b'] = bf(m['eye'])
    Pn = np.zeros((128, 257), f32)
    Pn[:, 128] = 1.0
    m['Pones'] = bf(Pn)
    sel = np.zeros((128, 128, 128), f32)
    for dd in range(128):
        sel[dd, dd, :] = 1.0
    m['sel'] = bf(sel)
    m['Jc'] = np.full((128, 128), 1.0 / C, f32)
    m['Jc_b'] = bf(m['Jc'])
    m['Jc_r'] = m['Jc'].copy()
    m['ones_row'] = np.ones((1, 512), f32)

    # ---- CAB ----
    c1w = np.asarray(d['cab_c1_w'], f32)                     # [O,I,3]
    m['c1wT'] = bf(c1w.transpose(1, 2, 0))                   # [I=128,3,O=128]
    m['c1b_row'] = cc(np.asarray(d['cab_c1_b'], f32)[None, :])
    m['ln1_g'] = cc(np.asarray(d['cab_ln1_g'], f32)[:, None])
    m['ln1_b'] = cc(np.asarray(d['cab_ln1_b'], f32)[:, None])
    m['qkwT'] = bf(np.concatenate([np.asarray(d['ca_q_w'], f32).T,
                                   np.asarray(d['ca_k_w'], f32).T], 1))
    m['qkb_row'] = cc(np.concatenate([np.asarray(d['ca_q_b'], f32),
                                      np.asarray(d['ca_k_b'], f32)])[None, :])
    c2w = np.asarray(d['cab_c2_w'], f32)
    m['c2wT'] = cc(c2w.transpose(1, 2, 0))
    m['c2b_row'] = cc(np.asarray(d['cab_c2_b'], f32)[None, :])
    m['ln2_g'] = cc(np.asarray(d['cab_ln2_g'], f32)[:, None])
    m['ln2_b'] = cc(np.asarray(d['cab_ln2_b'], f32)[:, None])

    # ---- mamba front ----
    m['mln_g'] = cc(np.asarray(d['m_ln_g'], f32)[:, None])
    m['mln_b'] = cc(np.asarray(d['m_ln_b'], f32)[:, None])
    order = np.concatenate([np.arange(q * 128, (q + 1) * 128),
                            np.arange((1 - q) * 128, (2 - q) * 128)])
    in_w = np.asarray(d['m_in_w'], f32)                      # [512, C]
    w_xi = in_w[:DI][order]
    w_z = in_w[DI:][order[:128]]
    m['in_wT'] = cc(np.concatenate([w_xi, w_z], 0).T)        # [C,384]
    cw = np.asarray(d['m_conv_w'], f32)[order]               # [256, 4]
    diag = np.zeros((128, 2 * DC, 128), f32)
    for h in range(2):
        for k in range(DC):
            np.fill_diagonal(diag[:, h * DC + k, :], cw[h * 128:(h + 1) * 128, k])
    m['conv_diag'] = bf(diag)                                # [128, 8, 128]
    cb = np.asarray(d['m_conv_b'], f32)[order]
    m['convb'] = cc(cb.reshape(2, 128).T)                    # [128, 2]
    xp_w = np.asarray(d['m_xproj_w'], f32)                   # [264, 256]
    xp_wT = xp_w[:, order].T                                 # [256, 264]
    # pack k-halves into free dim: [128, 2, X]
    m['xpw_dtl'] = bf(xp_wT[:, :R].reshape(2, 128, R).transpose(1, 0, 2))
    m['xpw_B'] = bf(xp_wT[:, R:R + S].reshape(2, 128, S).transpose(1, 0, 2))
    m['xpw_C'] = bf(xp_wT[:, R + S:].reshape(2, 128, S).transpose(1, 0, 2))
    dt_w = np.asarray(d['m_dt_w'], f32)                      # [256, 8]
    m['dtwT'] = cc(dt_w[order[:128]].T)                      # [8, 128]
    m['dtb_col'] = cc(np.asarray(d['m_dt_b'], f32)[order[:128], None])
    A = -np.exp(np.asarray(d['m_Alog'], f32))                # [256, 128]
    m['ATneg'] = cc(A[order[:128]].T)                        # [S, 128]
    m['Dcol'] = cc(np.asarray(d['m_D'], f32)[order[:128], None])
    out_w = np.asarray(d['m_out_w'], f32)                    # [C, 256]
    m['outwT'] = cc(out_w[:, order[:128]].T)                 # [128, C]

    # ---- MKGU ----
    m['kln_g'] = cc(np.asarray(d['k_ln_g'], f32)[:, None])
    m['kln_b'] = cc(np.asarray(d['k_ln_b'], f32)[:, None])
    kp_w = np.asarray(d['k_proj_w'], f32)                    # [2C, C]
    m['kpwT'] = cc(kp_w.T)                                   # [C, 256]
    m['kpb'] = cc(np.asarray(d['k_proj_b'], f32).reshape(2, 128).T)
    mc_comb = np.zeros((128, 31, 128), f32)
    for ci, p in enumerate([3, 7, 11, 15]):
        w = np.asarray(d[f'k_c{ci + 1}_w'], f32)             # [32, 128, 2p+1]
        for k in range(2 * p + 1):
            sft = k - p
            mc_comb[:, sft + 15, 32 * ci:32 * (ci + 1)] = w[:, :, k].T
    m['mc_comb'] = bf(mc_comb)
    mcb = np.concatenate([np.asarray(d[f'k_c{i}_b'], f32) for i in range(1, 5)])
    m['mcb_col'] = cc(mcb[:, None])
    dww = np.asarray(d['k_dw_w'], f32)[:, 0, :]              # [128, 31]
    dwd = np.zeros((128, 31, 128), f32)
    for k in range(31):
        np.fill_diagonal(dwd[:, k, :], dww[:, k])
    m['dw_diag'] = bf(dwd)                                   # [128, 31, 128]
    s1 = np.asarray(d['k_bn1_g'], f32) / np.sqrt(np.float32(1.0 + EPS))
    m['bn1_s'] = cc(s1[:, None])
    # silu(bn1_s*(dw+dw_b)+bn1_b) folds the dw-conv bias into the BN bias
    m['bn1_sb'] = cc((s1 * np.asarray(d['k_dw_b'], f32)
                      + np.asarray(d['k_bn1_b'], f32))[:, None])
    s2 = np.asarray(d['k_bn2_g'], f32) / np.sqrt(np.float32(1.0 + EPS))
    m['bn2_s'] = cc(s2[:, None]); m['bn2_b'] = cc(np.asarray(d['k_bn2_b'], f32)[:, None])
    m['pn_g'] = cc(np.asarray(d['pn_g'], f32)[:, None])
    m['pn_b'] = cc(np.asarray(d['pn_b'], f32)[:, None])
    mask = np.ones((128, EXT), f32)
    if q == 0:
        mask[:, :EXTL] = 0.0
    else:
        mask[:, EXT - EXTL:] = 0.0
    m['hmask'] = mask
    mcm = np.ones((128, MCW), f32)
    if q == 0:
        mcm[:, :16] = 0.0
    else:
        mcm[:, MCW - 16:] = 0.0
    m['mcmask'] = mcm
    return m


def gather_output(results):
    out = np.zeros((B, N, C), np.float32)
    for b in range(B):
        out[b, :NH] = results[2 * b]['out_half']
        out[b, NH:] = results[2 * b + 1]['out_half']
    return out


IN_SPECS = {
    'xT': ([C, N], F32), 'xr': ([C, N], BF16), 'eye': ([128, 128], F32),
    'eye_b': ([128, 128], BF16),
    'Pones': ([128, 257], BF16), 'sel': ([128, 128, 128], BF16),
    'Jc': ([128, 128], F32), 'Jc_b': ([128, 128], BF16),
    'Jc_r': ([128, 128], F32R),
    'ones_row': ([1, 512], F32R),
    'c1wT': ([128, 3, 128], BF16), 'c1b_row': ([1, 128], F32R),
    'ln1_g': ([128, 1], F32), 'ln1_b': ([128, 1], F32),
    'qkwT': ([128, 256], BF16), 'qkb_row': ([1, 256], F32R),
    'c2wT': ([128, 3, 128], F32R), 'c2b_row': ([1, 128], F32R),
    'ln2_g': ([128, 1], F32), 'ln2_b': ([128, 1], F32),
    'mln_g': ([128, 1], F32), 'mln_b': ([128, 1], F32),
    'in_wT': ([C, 384], F32R),
    'conv_diag': ([128, 8, 128], BF16), 'convb': ([128, 2], F32),
    'xpw_dtl': ([128, 2, R], BF16), 'xpw_B': ([128, 2, S], BF16),
    'xpw_C': ([128, 2, S], BF16),
    'dtwT': ([R, 128], F32R), 'dtb_col': ([128, 1], F32),
    'ATneg': ([S, 128], F32), 'Dcol': ([128, 1], F32), 'outwT': ([128, C], F32R),
    'kln_g': ([128, 1], F32), 'kln_b': ([128, 1], F32),
    'kpwT': ([C, 2 * C], F32R), 'kpb': ([128, 2], F32),
    'mc_comb': ([128, 31, 128], BF16),
    'mcb_col': ([128, 1], F32),
    'dw_diag': ([128, 31, 128], BF16),
    'bn1_s': ([128, 1], F32), 'bn1_sb': ([128, 1], F32),
    'bn2_s': ([128, 1], F32), 'bn2_b': ([128, 1], F32),
    'pn_g': ([128, 1], F32), 'pn_b': ([128, 1], F32),
    'hmask': ([128, EXT], F32), 'mcmask': ([128, MCW], F32),
}


def build(nc, debug=(), y_split=3):
    """Emit the kernel IR. debug: iterable of intermediate names to DMA out.
    y_split: every y_split-th d runs its y-mul on gpsimd (0 = never)."""
    I = {}
    for name, (shape, dt) in IN_SPECS.items():
        I[name] = nc.dram_tensor(name, shape, dt, kind="ExternalInput").ap()
    out_half = nc.dram_tensor("out_half", [NH, C], F32, kind="ExternalOutput").ap()

    rs_in_d = nc.dram_tensor("rs_in_d", [2, C, EXT], BF16).ap()
    dtx_d = nc.dram_tensor("dtx_d", [128, N], BF16).ap()
    rs_out_d = nc.dram_tensor("rs_out_d", [C, EXT], BF16).ap()
    groups = [[0, 1], [2, 3], [4, 5], [6, 7]]

    ctx = contextlib.ExitStack()
    tc = ctx.enter_context(tile.TileContext(nc, num_cores=N_CORES))
    persist = ctx.enter_context(tc.tile_pool(name="persist", bufs=1))
    work = ctx.enter_context(tc.tile_pool(name="work", bufs=1))
    wln = ctx.enter_context(tc.tile_pool(name="wln", bufs=1))
    front_ctx = contextlib.ExitStack()
    front_pool = front_ctx.enter_context(tc.tile_pool(name="front", bufs=1))
    cab_ctx = contextlib.ExitStack()
    cab_pool = cab_ctx.enter_context(tc.tile_pool(name="cab", bufs=1))
    ps_big = ctx.enter_context(tc.tile_pool(name="ps_big", bufs=2, space="PSUM"))
    ps_sm = ctx.enter_context(tc.tile_pool(name="ps_sm", bufs=2, space="PSUM"))
    # front-LN-only second stats pool: lets mean/msq of consecutive LN chunks
    # double-buffer independently; closed before ps_y claims its space
    msq_ctx = contextlib.ExitStack()
    ps_msq = msq_ctx.enter_context(tc.tile_pool(name="ps_msq", bufs=2,
                                                space="PSUM"))


    def dbg(name, ap):
        if name in debug:
            t = nc.dram_tensor("dbg_" + name, list(ap.shape), ap.dtype,
                               kind="ExternalOutput").ap()
            nc.sync.dma_start(out=t, in_=ap)

    CAB_INS = {'xT', 'xr', 'qkwT', 'qkb_row', 'eye_b', 'c1wT', 'c2wT',
               'c1b_row', 'c2b_row', 'ln1_g', 'ln1_b', 'ln2_g', 'ln2_b'}
    FRONT_INS = {'in_wT', 'conv_diag', 'xpw_B', 'xpw_C', 'xpw_dtl', 'dtwT',
                 'dtb_col', 'convb', 'mln_g', 'mln_b'}
    MKGU_INS = {'mc_comb', 'dw_diag', 'hmask', 'kpwT', 'kpb', 'mcb_col',
                'bn1_s', 'bn1_sb', 'bn2_s', 'bn2_b', 'kln_g', 'mcmask',
                'kln_b', 'pn_g', 'pn_b'}
    sb = {}

    def load_input(name, pool):
        shape, dt = IN_SPECS[name]
        tg = "cab_big" if name == 'xr' else name
        t = pool.tile(list(shape), dt, tag=tg, name="in_" + name)
        nc.sync.dma_start(out=t[:], in_=I[name])
        sb[name] = t

    prio = ['xr', 'c1wT', 'c1b_row', 'ones_row', 'Jc', 'Jc_r', 'ln1_g',
            'ln1_b', 'eye_b', 'qkwT', 'qkb_row', 'eye', 'Pones']
    rest = [n for n in IN_SPECS
            if n not in MKGU_INS and n != 'sel' and n not in prio]
    for name in prio + rest:
        load_input(name, cab_pool if name in CAB_INS else (
            front_pool if name in FRONT_INS else persist))
    # sel lives in persist so its SBUF exists from the start and its load
    # overlaps the CAB phase (pool creation/release must nest LIFO).
    sel_t = persist.tile([128, 128, 128], BF16, tag="sel", name="in_sel")
    for ch in range(4):
        nc.sync.dma_start(out=sel_t[:, ch * 32:(ch + 1) * 32, :],
                          in_=I['sel'][:, ch * 32:(ch + 1) * 32, :])
    sb['sel'] = sel_t

    zeros4 = persist.tile([128, 4], F32, tag="zeros4")
    nc.vector.memset(zeros4[:], 0.0)
    eps_col = persist.tile([128, 1], F32, tag="eps_col")
    nc.vector.memset(eps_col[:], EPS)
    one_col = persist.tile([128, 1], F32, tag="one_col")
    nc.vector.memset(one_col[:], 1.0)

    def mm(out, lhsT, rhs, start=True, stop=True):
        nc.tensor.matmul(out, lhsT, rhs, start=start, stop=stop)

    # ---- helpers ----------------------------------------------------------
    def ln_cpart(xT, g_col, b_col, width, relu=False, tag="ln", out_dt=F32,
                 out_tag=None, cw=256, msq_pool=None):
        """LayerNorm over the partition dim of xT [128, width].
        Emitted in cw-col chunks with chunk-local tiles so the serial
        mean/var/rstd/apply chain pipelines across chunks."""
        CW = cw
        outt = wln.tile([128, width], out_dt, tag=out_tag or ("ln_out_" + tag))
        for j0 in range(0, width, CW):
            j1 = min(j0 + CW, width)
            w = j1 - j0
            c = (slice(None), slice(j0, j1))
            xsq = wln.tile([128, CW], F32R, tag="ln_sq", bufs=3, name="ln_sq")
            nc.gpsimd.tensor_tensor(out=xsq[:, :w], in0=xT[c], in1=xT[c],
                                    op=AX.mult)
            mean_ps = ps_sm.tile([128, CW], F32, tag="psB", name="ln_mps")
            msq_ps = (msq_pool or ps_sm).tile([128, CW], F32, tag="psB2"
                      if msq_pool else "psB", name="ln_qps")
            jc = sb['Jc_b'] if xT.dtype == BF16 else sb['Jc']
            mm(mean_ps[:, :w], jc[:], xT[c])
            mm(msq_ps[:, :w], sb['Jc_r'][:], xsq[:, :w])
            mean = wln.tile([128, CW], F32, tag="ln_mean", bufs=3, name="ln_mean")
            nc.vector.tensor_copy(out=mean[:, :w], in_=mean_ps[:, :w])
            m2 = wln.tile([128, CW], F32, tag="ln_m2", bufs=3, name="ln_m2")
            nc.gpsimd.tensor_tensor(out=m2[:, :w], in0=mean[:, :w],
                                    in1=mean[:, :w], op=AX.mult)
            var = wln.tile([128, CW], F32, tag="ln_var", bufs=3, name="ln_var")
            nc.vector.tensor_tensor(out=var[:, :w], in0=msq_ps[:, :w],
                                    in1=m2[:, :w], op=AX.subtract)
            std = wln.tile([128, CW], F32, tag="ln_std", bufs=3, name="ln_std")
            nc.scalar.activation(out=std[:, :w], in_=var[:, :w], func=AF.Sqrt,
                                 bias=eps_col[:])
            rstd = wln.tile([128, CW], F32, tag="ln_rstd", bufs=3, name="ln_rstd")
            nc.vector.reciprocal(out=rstd[:, :w], in_=std[:, :w])
            xm = wln.tile([128, CW], F32, tag="ln_xm", bufs=3, name="ln_xm")
            nc.gpsimd.tensor_tensor(out=xm[:, :w], in0=xT[c], in1=mean[:, :w],
                                    op=AX.subtract)
            xn = wln.tile([128, CW], F32, tag="ln_xn", bufs=3, name="ln_xn")
            nc.vector.tensor_tensor(out=xn[:, :w], in0=xm[:, :w],
                                    in1=rstd[:, :w], op=AX.mult)
            if relu:
                nc.scalar.activation(out=outt[c], in_=xn[:, :w], func=AF.Relu,
                                     scale=g_col, bias=b_col)
            else:
                nc.vector.tensor_scalar(out=outt[c], in0=xn[:, :w],
                                        scalar1=g_col, scalar2=b_col,
                                        op0=AX.mult, op1=AX.add)
        return outt

    def silu_into(out_ap, in_ap, width, tag, eng=None):
        sg = work.tile([128, width], F32, tag="sg", name="sg_" + tag)
        nc.scalar.activation(out=sg[:], in_=in_ap, func=AF.Sigmoid)
        (eng or nc.vector).tensor_tensor(out=out_ap, in0=in_ap, in1=sg[:],
                                         op=AX.mult)

    def conv3(out_ps, wT3, xpad, width, bias_row=None):
        """out_ps[:,:width] = sum_k wT3[:,k,:].T @ xpad[:, k:k+width] (+bias)"""
        for j0 in range(0, width, 512):
            j1 = min(j0 + 512, width)
            for k in range(3):
                mm(out_ps[:, j0:j1], wT3[:, k, :], xpad[:, k + j0:k + j1],
                   start=(k == 0), stop=False)
            mm(out_ps[:, j0:j1], bias_row,
               sb['ones_row'][0:1, 0:j1 - j0], start=False, stop=True)

    # =======================================================================
    # Phase 1: CAB (duplicated within each pair)
    # =======================================================================
    xr_pad = cab_pool.tile([128, N + 2], BF16, tag="convpad_b")
    nc.gpsimd.tensor_copy(out=xr_pad[:, 0:1], in_=zeros4[:, 0:1])
    nc.gpsimd.tensor_copy(out=xr_pad[:, N + 1:N + 2], in_=zeros4[:, 0:1])
    nc.gpsimd.tensor_copy(out=xr_pad[:, 1:N + 1], in_=sb['xr'][:])
    c1_ps = ps_big.tile([128, N], F32, tag="psA")
    conv3(c1_ps, sb['c1wT'], xr_pad, N, bias_row=sb['c1b_row'][0:1, :])
    c1conv = work.tile([128, N], F32, tag="conv_out")
    for j0 in range(0, N, 512):
        nc.scalar.activation(out=c1conv[:, j0:j0 + 512],
                             in_=c1_ps[:, j0:j0 + 512], func=AF.Copy)
    dbg('c1conv', c1conv[:])
    c1T = ln_cpart(c1conv[:], sb['ln1_g'][:], sb['ln1_b'][:], N, relu=True,
                   tag="c1", out_dt=BF16, out_tag="ln_out_c1", msq_pool=ps_msq)
    dbg('c1T', c1T[:])

    # xs = raw reshape (N,C)->(C,N) of c1: 8 PE transposes of strided slices
    xs = cab_pool.tile([128, N], BF16, tag="cab_big")
    for nh in range(8):
        tpool = ps_sm if nh % 2 == 0 else ps_msq
        tp = tpool.tile([128, 128], BF16,
                        tag="psB" if nh % 2 == 0 else "psB2")
        src = bass.AP(tensor=c1T.tensor, offset=c1T.offset + nh,
                      ap=[[c1T.ap[0][0], 128], [8, 128]])
        nc.tensor.transpose(tp[:], src, sb['eye_b'][:])
        nc.scalar.activation(out=xs[:, nh * 128:(nh + 1) * 128], in_=tp[:],
                             func=AF.Copy)
    dbg('xs', xs[:])

    QT = cab_pool.tile([128, 8, 128], BF16)
    KT = cab_pool.tile([128, 8, 128], BF16)
    for i in range(8):
        tpool = ps_sm if i % 2 == 0 else ps_msq
        tp = tpool.tile([128, 256], F32,
                        tag="psB" if i % 2 == 0 else "psB2")
        mm(tp[:], xs[:, i * 128:(i + 1) * 128], sb['qkwT'][:],
           start=True, stop=False)
        mm(tp[:], sb['ones_row'][0:1, 0:128], sb['qkb_row'][:],
           start=False, stop=True)
        if i % 2 == 0:
            nc.vector.tensor_copy(out=QT[:, i, :], in_=tp[:, 0:128])
            nc.scalar.activation(out=KT[:, i, :], in_=tp[:, 128:256],
                                 func=AF.Copy)
        else:
            nc.scalar.activation(out=QT[:, i, :], in_=tp[:, 0:128],
                                 func=AF.Copy)
            nc.vector.tensor_copy(out=KT[:, i, :], in_=tp[:, 128:256])
    cc_ps = ps_sm.tile([128, 128], F32, tag="psB")
    for i in range(8):
        mm(cc_ps[:], QT[:, i, :], KT[:, i, :], start=(i == 0), stop=(i == 7))
    mx = work.tile([128, 1], F32, tag="sm_mx")
    nc.vector.tensor_reduce(out=mx[:], in_=cc_ps[:], axis=mybir.AxisListType.X,
                            op=AX.max)
    cc_sh = work.tile([128, 128], F32, tag="sm_sh")
    nc.vector.tensor_scalar(out=cc_sh[:], in0=cc_ps[:], scalar1=mx[:],
                            scalar2=None, op0=AX.subtract)
    cc_e = work.tile([128, 128], F32, tag="sm_e")
    nc.scalar.activation(out=cc_e[:], in_=cc_sh[:], func=AF.Exp)
    sm_s = work.tile([128, 1], F32, tag="sm_mx")
    nc.vector.tensor_reduce(out=sm_s[:], in_=cc_e[:], axis=mybir.AxisListType.X,
                            op=AX.add)
    sm_r = work.tile([128, 1], F32, tag="sm_r")
    nc.vector.reciprocal(out=sm_r[:], in_=sm_s[:])
    cc = cab_pool.tile([128, 128], BF16)
    nc.vector.tensor_scalar(out=cc[:], in0=cc_e[:], scalar1=sm_r[:],
                            scalar2=None, op0=AX.mult)
    dbg('cc', cc[:])

    xca = cab_pool.tile([128, N], F32, tag="cab_big")
    for j0 in range(0, N, 512):
        xca_ps = ps_sm.tile([128, 512], F32, tag="psB")
        mm(xca_ps[:], cc[:], c1T[:, j0:j0 + 512])
        nc.vector.scalar_tensor_tensor(out=xca[:, j0:j0 + 512],
                                       in0=c1T[:, j0:j0 + 512], scalar=2.0,
                                       in1=xca_ps[:], op0=AX.mult, op1=AX.add)
    xca_pad = cab_pool.tile([128, N + 2], F32R, tag="convpad")
    nc.gpsimd.tensor_copy(out=xca_pad[:, 0:1], in_=zeros4[:, 0:1])
    nc.gpsimd.tensor_copy(out=xca_pad[:, N + 1:N + 2], in_=zeros4[:, 0:1])
    nc.gpsimd.tensor_copy(out=xca_pad[:, 1:N + 1], in_=xca[:])
    c2_ps = ps_big.tile([128, N], F32, tag="psA")
    conv3(c2_ps, sb['c2wT'], xca_pad, N, bias_row=sb['c2b_row'][0:1, :])
    c2conv = work.tile([128, N], F32, tag="conv_out")
    for j0 in range(0, N, 512):
        nc.scalar.activation(out=c2conv[:, j0:j0 + 512],
                             in_=c2_ps[:, j0:j0 + 512], func=AF.Copy)
    c2T = ln_cpart(c2conv[:], sb['ln2_g'][:], sb['ln2_b'][:], N, relu=True,
                   tag="c2", out_tag="ln_out_seq", msq_pool=ps_msq)
    x2T = persist.tile([128, N], F32)
    for j0 in range(0, N, 512):
        nc.vector.tensor_tensor(out=x2T[:, j0:j0 + 512],
                                in0=sb['xT'][:, j0:j0 + 512],
                                in1=c2T[:, j0:j0 + 512], op=AX.add)
    dbg('x2T', x2T[:])
    cab_ctx.close()

    # =======================================================================
    # Phase 2: mamba front (duplicated within each pair)
    # =======================================================================
    xnT = ln_cpart(x2T[:], sb['mln_g'][:], sb['mln_b'][:], N, tag="mln",
                   out_dt=F32R, out_tag="ln_out_seq", msq_pool=ps_msq)
    msq_ctx.close()
    xi_t = [persist.tile([128, N], BF16, tag='xi0', name='xi0'),
            front_pool.tile([128, N], BF16, tag='xi1', name='xi1')]
    zT = front_pool.tile([128, N], F32)
    silu_z = persist.tile([128, N], F32)
    xpad_t = [front_pool.tile([128, N + 3], BF16, tag=f'xpad{i}', name=f'xpad{i}') for i in range(2)]
    for g in range(3):
        for j0 in range(0, N, 512):
            pj = ps_sm.tile([128, 512], F32, tag="psB")
            mm(pj[:], sb['in_wT'][:, g * 128:(g + 1) * 128], xnT[:, j0:j0 + 512])
            if (g + j0 // 512) % 2 == 0:
                dst_ap = (xpad_t[g][:, 3 + j0:3 + j0 + 512] if g < 2
                          else zT[:, j0:j0 + 512])
                nc.scalar.activation(out=dst_ap, in_=pj[:], func=AF.Copy)
            elif g < 2:
                nc.vector.tensor_copy(out=xpad_t[g][:, 3 + j0:3 + j0 + 512],
                                      in_=pj[:])
            else:
                nc.vector.tensor_copy(out=zT[:, j0:j0 + 512], in_=pj[:])
    nc.gpsimd.tensor_copy(out=xpad_t[0][:, 0:3], in_=zeros4[:, 0:3])
    nc.gpsimd.tensor_copy(out=xpad_t[1][:, 0:3], in_=zeros4[:, 0:3])
    for j0 in range(0, N, 512):
        sgz = work.tile([128, 512], F32, tag="sg", bufs=2, name="sg_z")
        nc.scalar.activation(out=sgz[:], in_=zT[:, j0:j0 + 512],
                             func=AF.Sigmoid)
        nc.gpsimd.tensor_tensor(out=silu_z[:, j0:j0 + 512],
                                in0=zT[:, j0:j0 + 512], in1=sgz[:],
                                op=AX.mult)
    for g in range(2):
        cps = ps_big.tile([128, N], F32, tag="psA")
        for j0 in range(0, N, 512):
            for k in range(DC):
                mm(cps[:, j0:j0 + 512], sb['conv_diag'][:, g * DC + k, :],
                   xpad_t[g][:, k + j0:k + j0 + 512],
                   start=(k == 0), stop=(k == 3))
            sgx = work.tile([128, 512], F32, tag="sg", bufs=2, name="sg_xi")
            nc.scalar.activation(out=sgx[:], in_=cps[:, j0:j0 + 512],
                                 func=AF.Sigmoid,
                                 bias=sb['convb'][:, g:g + 1])
            xc = work.tile([128, 512], F32, tag="xc_chunk", bufs=2,
                           name="xc")
            nc.vector.tensor_scalar(out=xc[:], in0=cps[:, j0:j0 + 512],
                                    scalar1=1.0,
                                    scalar2=sb['convb'][:, g:g + 1],
                                    op0=AX.mult, op1=AX.add)
            eng = nc.vector if g == 0 else nc.gpsimd
            eng.tensor_tensor(out=xi_t[g][:, j0:j0 + 512], in0=xc[:],
                              in1=sgx[:], op=AX.mult)
    dbg('xi0', xi_t[0][:])
    xi_r = xi_t
    dtl = front_pool.tile([8, N], F32R)
    BmT = persist.tile([S, N], BF16)
    CmT = persist.tile([S, N], BF16)
    cp_i = 0
    for (dst, wname, Msz) in ((dtl, 'xpw_dtl', R), (BmT, 'xpw_B', S),
                              (CmT, 'xpw_C', S)):
        for j0 in range(0, N, 512):
            pj = ps_sm.tile([Msz, 512], F32, tag="psB")
            for kk in range(2):
                mm(pj[:], sb[wname][:, kk, :], xi_r[kk][:, j0:j0 + 512],
                   start=(kk == 0), stop=(kk == 1))
            if cp_i % 2 == 0:
                nc.scalar.activation(out=dst[:, j0:j0 + 512], in_=pj[:],
                                     func=AF.Copy)
            else:
                nc.vector.tensor_copy(out=dst[:, j0:j0 + 512], in_=pj[:])
            cp_i += 1
    dbg('BmT', BmT[:]); dbg('CmT', CmT[:]); dbg('dtl', dtl[:])
    dtT = front_pool.tile([128, N], F32)
    for j0 in range(0, N, 512):
        pj = ps_sm.tile([128, 512], F32, tag="psB", name="pj_dt")
        mm(pj[:], sb['dtwT'][:], dtl[:, j0:j0 + 512])
        nc.scalar.activation(out=dtT[:, j0:j0 + 512], in_=pj[:],
                             func=AF.Exp, bias=sb['dtb_col'][:])
    for j0 in range(0, N, 512):
        nc.scalar.activation(out=dtT[:, j0:j0 + 512], in_=dtT[:, j0:j0 + 512],
                             func=AF.Ln, bias=one_col[:])
    dbg('dtT', dtT[:])
    dtb16 = persist.tile([128, N], BF16)
    for j0 in range(0, N, 512):
        nc.gpsimd.tensor_copy(out=dtb16[:, j0:j0 + 512],
                              in_=dtT[:, j0:j0 + 512])
    dtxT = front_pool.tile([128, N], BF16)
    for j0 in range(0, N, 512):
        nc.vector.tensor_tensor(out=dtxT[:, j0:j0 + 512],
                                in0=dtT[:, j0:j0 + 512],
                                in1=xi_t[0][:, j0:j0 + 512], op=AX.mult)
        nc.sync.dma_start(out=dtx_d[:, j0:j0 + 512], in_=dtxT[:, j0:j0 + 512])
    front_ctx.close()
    psy_ctx = contextlib.ExitStack()
    ps_y = psy_ctx.enter_context(tc.tile_pool(name="ps_y", bufs=1,
                                              space="PSUM"))
    scan_pool = ctx.enter_context(tc.tile_pool(name="scan", bufs=2))
    mkgu_pool = scan_pool
    # MKGU weight loads are chunked and interleaved into the scan loop below
    # so they ride SP's slack instead of stalling the post-scan phase.
    mkgu_loads = []
    for name in sorted(MKGU_INS):
        shape, dt = IN_SPECS[name]
        t = mkgu_pool.tile(list(shape), dt, tag=name, name="in_" + name,
                           bufs=1)
        sb[name] = t
        if len(shape) == 3 and shape[1] > 8:
            for k in range(0, shape[1], 4):
                k1 = min(k + 4, shape[1])
                mkgu_loads.append((t[:, k:k1, :], I[name][:, k:k1, :]))
        else:
            mkgu_loads.append((t[:], I[name]))

    # =======================================================================
    # Phase 3: selective scan over my 128 d's.
    # Per d: PE broadcasts the dt row into PSUM (ones[1,128] matmul from the
    # single-partition slice), Act exponentiates with the per-state A column,
    # a stride-0 DRAM DMA broadcasts the dtx row, and the b/scan/g elementwise
    # chain is split between DVE and Pool to balance engine load. The y
    # reduction over states stays on PE via the sliding one-hot.
    # =======================================================================
    y_ps = ps_y.tile([128, N], F32)
    for dd in range(128):
        dtx_bc = scan_pool.tile([128, N], BF16, tag="dtx_bc", bufs=5)
        src = bass.AP(tensor=dtx_d.tensor, offset=dd * N,
                      ap=[[0, 128], [1, N]])
        nc.sync.dma_start(out=dtx_bc[:], in_=src)
        dt_ps = ps_big.tile([128, N], F32, tag="psA", name="dtps")
        for j0 in range(0, N, 512):
            mm(dt_ps[:, j0:j0 + 512], sb['sel'][:, dd, :],
               dtb16[:, j0:j0 + 512])
        a_t = scan_pool.tile([128, N], BF16, tag="a", bufs=4)
        nc.scalar.activation(out=a_t[:], in_=dt_ps[:], func=AF.Exp,
                             scale=sb['ATneg'][:, dd:dd + 1])
        b_t = scan_pool.tile([128, N], BF16, tag="b", bufs=4)
        nc.gpsimd.tensor_tensor(out=b_t[:], in0=BmT[:], in1=dtx_bc[:],
                                op=AX.mult)
        h_t = scan_pool.tile([128, N], BF16, tag="h", bufs=4)
        nc.vector.tensor_tensor_scan(out=h_t[:], data0=a_t[:],
                                     data1=b_t[:], initial=0.0,
                                     op0=AX.mult, op1=AX.add)
        g_t = scan_pool.tile([128, N], BF16, tag="g", bufs=4)
        g_eng = nc.vector if dd % 5 < 2 else nc.gpsimd
        g_eng.tensor_tensor(out=g_t[:], in0=h_t[:], in1=CmT[:], op=AX.mult)
        for j0 in range(0, N, 512):
            mm(y_ps[:, j0:j0 + 512], sb['Pones'][:, 128 - dd:256 - dd],
               g_t[:, j0:j0 + 512], start=(dd == 0), stop=(dd == 127))
        if dd >= 8 and dd - 8 < len(mkgu_loads):
            ldst, lsrc = mkgu_loads[dd - 8]
            nc.sync.dma_start(out=ldst, in_=lsrc)

    yg = work.tile([128, N], F32, tag="mk_a", name="yg")
    ygate = persist.tile([128, N], F32R)
    for j0 in range(0, N, 512):
        nc.vector.scalar_tensor_tensor(out=yg[:, j0:j0 + 512],
                                       in0=xi_t[0][:, j0:j0 + 512],
                                       scalar=sb['Dcol'][:],
                                       in1=y_ps[:, j0:j0 + 512],
                                       op0=AX.mult, op1=AX.add)
        nc.gpsimd.tensor_tensor(out=ygate[:, j0:j0 + 512],
                                in0=yg[:, j0:j0 + 512],
                                in1=silu_z[:, j0:j0 + 512], op=AX.mult)
    dbg('yscan', yg[:])
    psy_ctx.close()
    ps_tail = ctx.enter_context(tc.tile_pool(name="ps_tail", bufs=2,
                                             space="PSUM"))
    op_ps = ps_big.tile([128, N], F32, tag="psA")
    for j0 in range(0, N, 512):
        mm(op_ps[:, j0:j0 + 512], sb['outwT'][:], ygate[:, j0:j0 + 512])
    rs_in = persist.tile([128, 2 * EXT], BF16)
    nc.vector.memset(rs_in[:, 0:EXTL], 0.0)
    nc.vector.memset(rs_in[:, 2 * EXT - EXTL:], 0.0)
    nc.vector.scalar_tensor_tensor(out=rs_in[:, EXTL:EXT],
                                   in0=x2T[:, 0:EXT - EXTL], scalar=0.5,
                                   in1=op_ps[:, 0:EXT - EXTL],
                                   op0=AX.mult, op1=AX.add)
    nc.vector.scalar_tensor_tensor(out=rs_in[:, EXT:2 * EXT - EXTL],
                                   in0=x2T[:, NH - EXTL:N], scalar=0.5,
                                   in1=op_ps[:, NH - EXTL:N],
                                   op0=AX.mult, op1=AX.add)
    nc.sync.dma_start(out=rs_in_d[0], in_=rs_in[:, 0:EXT])
    nc.scalar.dma_start(out=rs_in_d[1], in_=rs_in[:, EXT:])
    nc.gpsimd.collective_compute("ReduceScatter", AX.add, replica_groups=groups,
                                 ins=[rs_in_d], outs=[rs_out_d])
    warm_ps = ps_sm.tile([128, 512], F32, tag="psB", name="warm")
    for _w in range(56):
        mm(warm_ps[:], sb['Pones'][:, 0:128], rs_in[:, 0:512],
           start=(_w == 0), stop=(_w == 55))
    x3e = persist.tile([128, EXT], BF16)
    nc.sync.dma_start(out=x3e[:, 0:288], in_=rs_out_d[:, 0:288])
    nc.scalar.dma_start(out=x3e[:, 288:], in_=rs_out_d[:, 288:])
    dbg('x3e', x3e[:])

    # =======================================================================
    # Phase 4: MKGU on my region
    # =======================================================================
    knT = ln_cpart(x3e[:], sb['kln_g'][:], sb['kln_b'][:], EXT, tag="kln",
                   out_dt=F32R, msq_pool=ps_tail)
    x_dc = mkgu_pool.tile([128, EXT], F32)
    x_mc = mkgu_pool.tile([128, EXT], BF16)
    for g in (1, 0):
        hp = ps_big.tile([128, EXT], F32, tag="psA")
        for j0 in range(0, EXT, 512):
            j1 = min(j0 + 512, EXT)
            mm(hp[:, j0:j1], sb['kpwT'][:, g * 128:(g + 1) * 128], knT[:, j0:j1])
        sg = work.tile([128, EXT], F32, tag="sg_h")
        nc.scalar.activation(out=sg[:], in_=hp[:], func=AF.Sigmoid,
                             bias=sb['kpb'][:, g:g + 1])
        hs = work.tile([128, EXT], F32, tag="hpre")
        nc.vector.tensor_scalar(out=hs[:], in0=hp[:], scalar1=1.0,
                                scalar2=sb['kpb'][:, g:g + 1],
                                op0=AX.mult, op1=AX.add)
        hsw = work.tile([128, EXT], F32, tag="hsw")
        nc.vector.tensor_tensor(out=hsw[:], in0=hs[:], in1=sg[:], op=AX.mult)
        dst, eng = ((x_dc, nc.vector) if g == 0 else (x_mc, nc.gpsimd))
        eng.tensor_tensor(out=dst[:], in0=hsw[:], in1=sb['hmask'][:],
                          op=AX.mult)
    dbg('xmc', x_mc[:])
    mc_ps = ps_big.tile([128, MCW], F32, tag="psA")
    for j0 in range(0, MCW, 512):
        j1 = min(j0 + 512, MCW)
        for t in range(31):
            mm(mc_ps[:, j0:j1], sb['mc_comb'][:, t, :],
               x_mc[:, t + 1 + j0:t + 1 + j1], start=(t == 0), stop=(t == 30))
    mcT = mkgu_pool.tile([128, MCW], BF16)
    nc.vector.scalar_tensor_tensor(out=mcT[:], in0=mc_ps[:],
                                   scalar=sb['mcb_col'][:], in1=sb['mcmask'][:],
                                   op0=AX.add, op1=AX.mult)
    dbg('mc', mcT[:])
    dw_ps = ps_big.tile([128, NH], F32, tag="psA")
    for k in range(31):
        mm(dw_ps[:], sb['dw_diag'][:, k, :], mcT[:, k + 1:k + 1 + NH],
           start=(k == 0), stop=(k == 30))
    sg1 = work.tile([128, NH], F32, tag="sg_dw")
    nc.scalar.activation(out=sg1[:], in_=dw_ps[:], func=AF.Sigmoid,
                         scale=sb['bn1_s'][:], bias=sb['bn1_sb'][:])
    bn1 = work.tile([128, NH], F32, tag="mk_a")
    nc.vector.tensor_scalar(out=bn1[:], in0=dw_ps[:], scalar1=sb['bn1_s'][:],
                            scalar2=sb['bn1_sb'][:], op0=AX.mult, op1=AX.add)
    dw_silu = work.tile([128, NH], F32, tag="mk_b")
    nc.gpsimd.tensor_tensor(out=dw_silu[:], in0=bn1[:], in1=sg1[:], op=AX.mult)
    dwmc = work.tile([128, NH], F32, tag="mk_a")
    nc.gpsimd.tensor_tensor(out=dwmc[:], in0=dw_silu[:],
                            in1=mcT[:, 16:16 + NH], op=AX.add)
    warm2 = ps_tail.tile([128, 256], F32, tag="psB2", name="warm2")
    for _w in range(8):
        mm(warm2[:], sb['eye'][:], dwmc[:, 0:256],
           start=(_w == 0), stop=(_w == 7))
    sg2 = work.tile([128, NH], F32, tag="sg_dw")
    nc.scalar.activation(out=sg2[:], in_=dwmc[:], func=AF.Sigmoid,
                         scale=sb['bn2_s'][:], bias=sb['bn2_b'][:])
    bn2 = work.tile([128, NH], F32, tag="mk_b")
    nc.vector.tensor_scalar(out=bn2[:], in0=dwmc[:], scalar1=sb['bn2_s'][:],
                            scalar2=sb['bn2_b'][:], op0=AX.mult, op1=AX.add)
    bn2s = work.tile([128, NH], F32, tag="mk_c")
    nc.gpsimd.tensor_tensor(out=bn2s[:], in0=bn2[:], in1=sg2[:], op=AX.mult)
    outc = work.tile([128, NH], F32, tag="mk_b")
    nc.vector.tensor_tensor(out=outc[:], in0=bn2s[:],
                            in1=x_dc[:, EXTL:EXTL + NH], op=AX.mult)
    x4 = work.tile([128, NH], F32, tag="mk_a")
    nc.gpsimd.tensor_tensor(out=x4[:], in0=outc[:], in1=x3e[:, EXTL:EXTL + NH],
                            op=AX.add)
    x4n = ln_cpart(x4[:], sb['pn_g'][:], sb['pn_b'][:], NH, tag="pn",
                   msq_pool=ps_tail)
    for j in range(4):
        tp = ps_sm.tile([128, 128], F32, tag="psB")
        nc.tensor.transpose(tp[:], x4n[:, j * 128:(j + 1) * 128], sb['eye'][:])
        ot = work.tile([128, 128], F32, tag="out_sb", bufs=4)
        nc.vector.tensor_copy(out=ot[:], in_=tp[:])
        qeng = nc.sync if j % 2 == 0 else nc.scalar
        qeng.dma_start(out=out_half[j * 128:(j + 1) * 128, :], in_=ot[:])

    ctx.close()
    return nc


# --------------------------------------------------------------------------
# Entry point
# --------------------------------------------------------------------------
_CACHE = {}


def _get_nc():
    if "nc" not in _CACHE:
        nc = bacc.Bacc("TRN2", target_bir_lowering=False, debug=False,
                       num_devices=N_CORES)
        build(nc)
        nc.finalize()
        _CACHE["nc"] = nc
    return _CACHE["nc"]


def kernel(**inputs):
    import numpy as np
    nc = _get_nc()
    d = {k: np.asarray(v) for k, v in inputs.items()}
    in_maps = [prep_core_inputs(d, c // 2, c % 2) for c in range(N_CORES)]
    res = run_bass_kernel_spmd(nc, in_maps, core_ids=list(range(N_CORES)))
    return gather_output(res.results)



# revision 82
# speedup vs baseline: 1.0006x; 1.0006x over previous
"""Trainium2 Bass kernel for nn_CMKConMambaBlock (ConMamba block).

Sharding: 8 NeuronCores = 4 batch x 2 d_inner-halves. Single SPMD program;
per-core differences are injected purely through host-prepared inputs
(weights reordered so each core's d-half is local slice 0; the MKGU
N-region is routed by a pairwise ReduceScatter). The Mamba selective scan
runs as 128 per-d tensor_tensor_scan recurrences in an
[s=128 partitions, t=1024 free] layout. Per d, the dt row is broadcast on
the PE via a per-d one-hot selector matmul into PSUM, Act exponentiates it
with the per-state A column, the dtx row is broadcast by a stride-0 DRAM
DMA on the otherwise-idle SP queue, and the b/scan/g elementwise chain is
split across DVE and Pool (scan itself is DVE-only on HW; Pool never
touches PSUM). The y reduction over states stays on PE via the sliding
one-hot. MKGU weights stream in on SP slack chunked inside the scan loop;
conv biases ride the PE accumulation as rank-1 ones-row taps; the
ReduceScatter payload is bf16. Elementwise work is spread across
DVE/Pool/Act per the legacy cost model rates (DVE 2x for packed bf16,
Act/Pool dtype-agnostic).

kernel(**inputs) takes the full unsharded inputs (as produced by
setup_inputs()) and returns the full (4, 1024, 128) float32 output.
"""
import sys
for _p in ("/opt/trn_rl_repo", "/root/.axon_site/_ro/trn_rl_repo"):
    if _p not in sys.path:
        sys.path.append(_p)


import contextlib
import numpy as np
import ml_dtypes

import concourse.bass as bass
import concourse.bacc as bacc
from concourse.bass_utils import run_bass_kernel_spmd
import concourse.tile as tile
from concourse import mybir

F32 = mybir.dt.float32
F32R = mybir.dt.float32r
BF16 = mybir.dt.bfloat16
AX = mybir.AluOpType
AF = mybir.ActivationFunctionType

B, N, C = 4, 1024, 128
DI, S, R, DC = 256, 128, 8, 4
EPS = 1e-5
NH = 512          # N half
EXTL = 32         # halo for MKGU region
EXT = NH + 2 * EXTL   # 576
MCW = NH + 32     # mc width needed for dw conv: [n0-16, n0+528)
N_CORES = 8
BF = ml_dtypes.bfloat16


# --------------------------------------------------------------------------
# Host-side input prep (numpy only; layout + weight reorg).
# --------------------------------------------------------------------------
def prep_core_inputs(d, b, q):
    """d: dict of full inputs (numpy). Returns in_map for core 2*b+q."""
    f32 = np.float32

    def bf(x):
        return np.ascontiguousarray(np.asarray(x, f32).astype(BF))

    def cc(x):
        return np.ascontiguousarray(np.asarray(x, f32))

    m = {}
    x = np.asarray(d['x'], f32)
    m['xT'] = cc(x[b].T)                                     # [C,N]
    m['xr'] = bf(x[b].reshape(C, N))                         # raw reshape
    m['eye'] = np.eye(128, dtype=f32)
    m['eye_b'] = bf(m['eye'])
    Pn = np.zeros((128, 257), f32)
    Pn[:, 128] = 1.0
    m['Pones'] = bf(Pn)
    sel = np.zeros((128, 128, 128), f32)
    for dd in range(128):
        sel[dd, dd, :] = 1.0
    m['sel'] = bf(sel)
    m['Jc'] = np.full((128, 128), 1.0 / C, f32)
    m['Jc_b'] = bf(m['Jc'])
    m['Jc_r'] = m['Jc'].copy()
    m['ones_row'] = np.ones((1, 512), f32)

    # ---- CAB ----
    c1w = np.asarray(d['cab_c1_w'], f32)                     # [O,I,3]
    m['c1wT'] = bf(c1w.transpose(1, 2, 0))                   # [I=128,3,O=128]
    m['c1b_row'] = cc(np.asarray(d['cab_c1_b'], f32)[None, :])
    m['ln1_g'] = cc(np.asarray(d['cab_ln1_g'], f32)[:, None])
    m['ln1_b'] = cc(np.asarray(d['cab_ln1_b'], f32)[:, None])
    m['qkwT'] = bf(np.concatenate([np.asarray(d['ca_q_w'], f32).T,
                                   np.asarray(d['ca_k_w'], f32).T], 1))
    m['qkb_row'] = cc(np.concatenate([np.asarray(d['ca_q_b'], f32),
                                      np.asarray(d['ca_k_b'], f32)])[None, :])
    c2w = np.asarray(d['cab_c2_w'], f32)
    m['c2wT'] = cc(c2w.transpose(1, 2, 0))
    m['c2b_row'] = cc(np.asarray(d['cab_c2_b'], f32)[None, :])
    m['ln2_g'] = cc(np.asarray(d['cab_ln2_g'], f32)[:, None])
    m['ln2_b'] = cc(np.asarray(d['cab_ln2_b'], f32)[:, None])

    # ---- mamba front ----
    m['mln_g'] = cc(np.asarray(d['m_ln_g'], f32)[:, None])
    m['mln_b'] = cc(np.asarray(d['m_ln_b'], f32)[:, None])
    order = np.concatenate([np.arange(q * 128, (q + 1) * 128),
                            np.arange((1 - q) * 128, (2 - q) * 128)])
    in_w = np.asarray(d['m_in_w'], f32)                      # [512, C]
    w_xi = in_w[:DI][order]
    w_z = in_w[DI:][order[:128]]
    m['in_wT'] = cc(np.concatenate([w_xi, w_z], 0).T)        # [C,384]
    cw = np.asarray(d['m_conv_w'], f32)[order]               # [256, 4]
    diag = np.zeros((128, 2 * DC, 128), f32)
    for h in range(2):
        for k in range(DC):
            np.fill_diagonal(diag[:, h * DC + k, :], cw[h * 128:(h + 1) * 128, k])
    m['conv_diag'] = bf(diag)                                # [128, 8, 128]
    cb = np.asarray(d['m_conv_b'], f32)[order]
    m['convb'] = cc(cb.reshape(2, 128).T)                    # [128, 2]
    xp_w = np.asarray(d['m_xproj_w'], f32)                   # [264, 256]
    xp_wT = xp_w[:, order].T                                 # [256, 264]
    # pack k-halves into free dim: [128, 2, X]
    m['xpw_dtl'] = bf(xp_wT[:, :R].reshape(2, 128, R).transpose(1, 0, 2))
    m['xpw_B'] = bf(xp_wT[:, R:R + S].reshape(2, 128, S).transpose(1, 0, 2))
    m['xpw_C'] = bf(xp_wT[:, R + S:].reshape(2, 128, S).transpose(1, 0, 2))
    dt_w = np.asarray(d['m_dt_w'], f32)                      # [256, 8]
    m['dtwT'] = cc(dt_w[order[:128]].T)                      # [8, 128]
    m['dtb_col'] = cc(np.asarray(d['m_dt_b'], f32)[order[:128], None])
    A = -np.exp(np.asarray(d['m_Alog'], f32))                # [256, 128]
    m['ATneg'] = cc(A[order[:128]].T)                        # [S, 128]
    m['Dcol'] = cc(np.asarray(d['m_D'], f32)[order[:128], None])
    out_w = np.asarray(d['m_out_w'], f32)                    # [C, 256]
    m['outwT'] = cc(out_w[:, order[:128]].T)                 # [128, C]

    # ---- MKGU ----
    m['kln_g'] = cc(np.asarray(d['k_ln_g'], f32)[:, None])
    m['kln_b'] = cc(np.asarray(d['k_ln_b'], f32)[:, None])
    kp_w = np.asarray(d['k_proj_w'], f32)                    # [2C, C]
    m['kpwT'] = cc(kp_w.T)                                   # [C, 256]
    m['kpb'] = cc(np.asarray(d['k_proj_b'], f32).reshape(2, 128).T)
    mc_comb = np.zeros((128, 31, 128), f32)
    for ci, p in enumerate([3, 7, 11, 15]):
        w = np.asarray(d[f'k_c{ci + 1}_w'], f32)             # [32, 128, 2p+1]
        for k in range(2 * p + 1):
            sft = k - p
            mc_comb[:, sft + 15, 32 * ci:32 * (ci + 1)] = w[:, :, k].T
    m['mc_comb'] = bf(mc_comb)
    mcb = np.concatenate([np.asarray(d[f'k_c{i}_b'], f32) for i in range(1, 5)])
    m['mcb_col'] = cc(mcb[:, None])
    dww = np.asarray(d['k_dw_w'], f32)[:, 0, :]              # [128, 31]
    dwd = np.zeros((128, 31, 128), f32)
    for k in range(31):
        np.fill_diagonal(dwd[:, k, :], dww[:, k])
    m['dw_diag'] = bf(dwd)                                   # [128, 31, 128]
    s1 = np.asarray(d['k_bn1_g'], f32) / np.sqrt(np.float32(1.0 + EPS))
    m['bn1_s'] = cc(s1[:, None])
    # silu(bn1_s*(dw+dw_b)+bn1_b) folds the dw-conv bias into the BN bias
    m['bn1_sb'] = cc((s1 * np.asarray(d['k_dw_b'], f32)
                      + np.asarray(d['k_bn1_b'], f32))[:, None])
    s2 = np.asarray(d['k_bn2_g'], f32) / np.sqrt(np.float32(1.0 + EPS))
    m['bn2_s'] = cc(s2[:, None]); m['bn2_b'] = cc(np.asarray(d['k_bn2_b'], f32)[:, None])
    m['pn_g'] = cc(np.asarray(d['pn_g'], f32)[:, None])
    m['pn_b'] = cc(np.asarray(d['pn_b'], f32)[:, None])
    mask = np.ones((128, EXT), f32)
    if q == 0:
        mask[:, :EXTL] = 0.0
    else:
        mask[:, EXT - EXTL:] = 0.0
    m['hmask'] = mask
    mcm = np.ones((128, MCW), f32)
    if q == 0:
        mcm[:, :16] = 0.0
    else:
        mcm[:, MCW - 16:] = 0.0
    m['mcmask'] = mcm
    return m


def gather_output(results):
    out = np.zeros((B, N, C), np.float32)
    for b in range(B):
        out[b, :NH] = results[2 * b]['out_half']
        out[b, NH:] = results[2 * b + 1]['out_half']
    return out


IN_SPECS = {
    'xT': ([C, N], F32), 'xr': ([C, N], BF16), 'eye': ([128, 128], F32),
    'eye_b': ([128, 128], BF16),
    'Pones': ([128, 257], BF16), 'sel': ([128, 128, 128], BF16),
    'Jc': ([128, 128], F32), 'Jc_b': ([128, 128], BF16),
    'Jc_r': ([128, 128], F32R),
    'ones_row': ([1, 512], F32R),
    'c1wT': ([128, 3, 128], BF16), 'c1b_row': ([1, 128], F32R),
    'ln1_g': ([128, 1], F32), 'ln1_b': ([128, 1], F32),
    'qkwT': ([128, 256], BF16), 'qkb_row': ([1, 256], F32R),
    'c2wT': ([128, 3, 128], F32R), 'c2b_row': ([1, 128], F32R),
    'ln2_g': ([128, 1], F32), 'ln2_b': ([128, 1], F32),
    'mln_g': ([128, 1], F32), 'mln_b': ([128, 1], F32),
    'in_wT': ([C, 384], F32R),
    'conv_diag': ([128, 8, 128], BF16), 'convb': ([128, 2], F32),
    'xpw_dtl': ([128, 2, R], BF16), 'xpw_B': ([128, 2, S], BF16),
    'xpw_C': ([128, 2, S], BF16),
    'dtwT': ([R, 128], F32R), 'dtb_col': ([128, 1], F32),
    'ATneg': ([S, 128], F32), 'Dcol': ([128, 1], F32), 'outwT': ([128, C], F32R),
    'kln_g': ([128, 1], F32), 'kln_b': ([128, 1], F32),
    'kpwT': ([C, 2 * C], F32R), 'kpb': ([128, 2], F32),
    'mc_comb': ([128, 31, 128], BF16),
    'mcb_col': ([128, 1], F32),
    'dw_diag': ([128, 31, 128], BF16),
    'bn1_s': ([128, 1], F32), 'bn1_sb': ([128, 1], F32),
    'bn2_s': ([128, 1], F32), 'bn2_b': ([128, 1], F32),
    'pn_g': ([128, 1], F32), 'pn_b': ([128, 1], F32),
    'hmask': ([128, EXT], F32), 'mcmask': ([128, MCW], F32),
}


def build(nc, debug=(), y_split=3):
    """Emit the kernel IR. debug: iterable of intermediate names to DMA out.
    y_split: every y_split-th d runs its y-mul on gpsimd (0 = never)."""
    I = {}
    for name, (shape, dt) in IN_SPECS.items():
        I[name] = nc.dram_tensor(name, shape, dt, kind="ExternalInput").ap()
    out_half = nc.dram_tensor("out_half", [NH, C], F32, kind="ExternalOutput").ap()

    rs_in_d = nc.dram_tensor("rs_in_d", [2, C, EXT], BF16).ap()
    dtx_d = nc.dram_tensor("dtx_d", [128, N], BF16).ap()
    rs_out_d = nc.dram_tensor("rs_out_d", [C, EXT], BF16).ap()
    groups = [[0, 1], [2, 3], [4, 5], [6, 7]]

    ctx = contextlib.ExitStack()
    tc = ctx.enter_context(tile.TileContext(nc, num_cores=N_CORES))
    persist = ctx.enter_context(tc.tile_pool(name="persist", bufs=1))
    work = ctx.enter_context(tc.tile_pool(name="work", bufs=1))
    wln = ctx.enter_context(tc.tile_pool(name="wln", bufs=1))
    front_ctx = contextlib.ExitStack()
    front_pool = front_ctx.enter_context(tc.tile_pool(name="front", bufs=1))
    cab_ctx = contextlib.ExitStack()
    cab_pool = cab_ctx.enter_context(tc.tile_pool(name="cab", bufs=1))
    ps_big = ctx.enter_context(tc.tile_pool(name="ps_big", bufs=2, space="PSUM"))
    ps_sm = ctx.enter_context(tc.tile_pool(name="ps_sm", bufs=2, space="PSUM"))
    # front-LN-only second stats pool: lets mean/msq of consecutive LN chunks
    # double-buffer independently; closed before ps_y claims its space
    msq_ctx = contextlib.ExitStack()
    ps_msq = msq_ctx.enter_context(tc.tile_pool(name="ps_msq", bufs=2,
                                                space="PSUM"))


    def dbg(name, ap):
        if name in debug:
            t = nc.dram_tensor("dbg_" + name, list(ap.shape), ap.dtype,
                               kind="ExternalOutput").ap()
            nc.sync.dma_start(out=t, in_=ap)

    CAB_INS = {'xT', 'xr', 'qkwT', 'qkb_row', 'eye_b', 'c1wT', 'c2wT',
               'c1b_row', 'c2b_row', 'ln1_g', 'ln1_b', 'ln2_g', 'ln2_b'}
    FRONT_INS = {'in_wT', 'conv_diag', 'xpw_B', 'xpw_C', 'xpw_dtl', 'dtwT',
                 'dtb_col', 'convb', 'mln_g', 'mln_b'}
    MKGU_INS = {'mc_comb', 'dw_diag', 'hmask', 'kpwT', 'kpb', 'mcb_col',
                'bn1_s', 'bn1_sb', 'bn2_s', 'bn2_b', 'kln_g', 'mcmask',
                'kln_b', 'pn_g', 'pn_b'}
    sb = {}

    def load_input(name, pool):
        shape, dt = IN_SPECS[name]
        tg = "cab_big" if name == 'xr' else name
        t = pool.tile(list(shape), dt, tag=tg, name="in_" + name)
        nc.sync.dma_start(out=t[:], in_=I[name])
        sb[name] = t

    prio = ['xr', 'c1wT', 'c1b_row', 'ones_row', 'Jc', 'Jc_r', 'ln1_g',
            'ln1_b', 'eye_b', 'qkwT', 'qkb_row', 'eye', 'Pones']
    rest = [n for n in IN_SPECS
            if n not in MKGU_INS and n != 'sel' and n not in prio]
    for name in prio + rest:
        load_input(name, cab_pool if name in CAB_INS else (
            front_pool if name in FRONT_INS else persist))
    # sel lives in persist so its SBUF exists from the start and its load
    # overlaps the CAB phase (pool creation/release must nest LIFO).
    sel_t = persist.tile([128, 128, 128], BF16, tag="sel", name="in_sel")
    for ch in range(4):
        nc.sync.dma_start(out=sel_t[:, ch * 32:(ch + 1) * 32, :],
                          in_=I['sel'][:, ch * 32:(ch + 1) * 32, :])
    sb['sel'] = sel_t

    zeros4 = persist.tile([128, 4], F32, tag="zeros4")
    nc.vector.memset(zeros4[:], 0.0)
    eps_col = persist.tile([128, 1], F32, tag="eps_col")
    nc.vector.memset(eps_col[:], EPS)
    one_col = persist.tile([128, 1], F32, tag="one_col")
    nc.vector.memset(one_col[:], 1.0)

    def mm(out, lhsT, rhs, start=True, stop=True):
        nc.tensor.matmul(out, lhsT, rhs, start=start, stop=stop)

    # ---- helpers ----------------------------------------------------------
    def ln_cpart(xT, g_col, b_col, width, relu=False, tag="ln", out_dt=F32,
                 out_tag=None, cw=256, msq_pool=None):
        """LayerNorm over the partition dim of xT [128, width].
        Emitted in cw-col chunks with chunk-local tiles so the serial
        mean/var/rstd/apply chain pipelines across chunks."""
        CW = cw
        outt = wln.tile([128, width], out_dt, tag=out_tag or ("ln_out_" + tag))
        for j0 in range(0, width, CW):
            j1 = min(j0 + CW, width)
            w = j1 - j0
            c = (slice(None), slice(j0, j1))
            xsq = wln.tile([128, CW], F32R, tag="ln_sq", bufs=2, name="ln_sq")
            nc.gpsimd.tensor_tensor(out=xsq[:, :w], in0=xT[c], in1=xT[c],
                                    op=AX.mult)
            mean_ps = ps_sm.tile([128, CW], F32, tag="psB", name="ln_mps")
            msq_ps = (msq_pool or ps_sm).tile([128, CW], F32, tag="psB2"
                      if msq_pool else "psB", name="ln_qps")
            jc = sb['Jc_b'] if xT.dtype == BF16 else sb['Jc']
            mm(mean_ps[:, :w], jc[:], xT[c])
            mm(msq_ps[:, :w], sb['Jc_r'][:], xsq[:, :w])
            mean = wln.tile([128, CW], F32, tag="ln_mean", bufs=2, name="ln_mean")
            nc.vector.tensor_copy(out=mean[:, :w], in_=mean_ps[:, :w])
            m2 = wln.tile([128, CW], F32, tag="ln_m2", bufs=2, name="ln_m2")
            nc.gpsimd.tensor_tensor(out=m2[:, :w], in0=mean[:, :w],
                                    in1=mean[:, :w], op=AX.mult)
            var = wln.tile([128, CW], F32, tag="ln_var", bufs=2, name="ln_var")
            nc.vector.tensor_tensor(out=var[:, :w], in0=msq_ps[:, :w],
                                    in1=m2[:, :w], op=AX.subtract)
            std = wln.tile([128, CW], F32, tag="ln_std", bufs=2, name="ln_std")
            nc.scalar.activation(out=std[:, :w], in_=var[:, :w], func=AF.Sqrt,
                                 bias=eps_col[:])
            rstd = wln.tile([128, CW], F32, tag="ln_rstd", bufs=2, name="ln_rstd")
            nc.vector.reciprocal(out=rstd[:, :w], in_=std[:, :w])
            xm = wln.tile([128, CW], F32, tag="ln_xm", bufs=2, name="ln_xm")
            nc.gpsimd.tensor_tensor(out=xm[:, :w], in0=xT[c], in1=mean[:, :w],
                                    op=AX.subtract)
            xn = wln.tile([128, CW], F32, tag="ln_xn", bufs=2, name="ln_xn")
            nc.vector.tensor_tensor(out=xn[:, :w], in0=xm[:, :w],
                                    in1=rstd[:, :w], op=AX.mult)
            if relu:
                nc.scalar.activation(out=outt[c], in_=xn[:, :w], func=AF.Relu,
                                     scale=g_col, bias=b_col)
            else:
                nc.vector.tensor_scalar(out=outt[c], in0=xn[:, :w],
                                        scalar1=g_col, scalar2=b_col,
                                        op0=AX.mult, op1=AX.add)
        return outt

    def silu_into(out_ap, in_ap, width, tag, eng=None):
        sg = work.tile([128, width], F32, tag="sg", name="sg_" + tag)
        nc.scalar.activation(out=sg[:], in_=in_ap, func=AF.Sigmoid)
        (eng or nc.vector).tensor_tensor(out=out_ap, in0=in_ap, in1=sg[:],
                                         op=AX.mult)

    def conv3(out_ps, wT3, xpad, width, bias_row=None):
        """out_ps[:,:width] = sum_k wT3[:,k,:].T @ xpad[:, k:k+width] (+bias)"""
        for j0 in range(0, width, 512):
            j1 = min(j0 + 512, width)
            for k in range(3):
                mm(out_ps[:, j0:j1], wT3[:, k, :], xpad[:, k + j0:k + j1],
                   start=(k == 0), stop=False)
            mm(out_ps[:, j0:j1], bias_row,
               sb['ones_row'][0:1, 0:j1 - j0], start=False, stop=True)

    # =======================================================================
    # Phase 1: CAB (duplicated within each pair)
    # =======================================================================
    xr_pad = cab_pool.tile([128, N + 2], BF16, tag="convpad_b")
    nc.gpsimd.tensor_copy(out=xr_pad[:, 0:1], in_=zeros4[:, 0:1])
    nc.gpsimd.tensor_copy(out=xr_pad[:, N + 1:N + 2], in_=zeros4[:, 0:1])
    nc.gpsimd.tensor_copy(out=xr_pad[:, 1:N + 1], in_=sb['xr'][:])
    c1_ps = ps_big.tile([128, N], F32, tag="psA")
    conv3(c1_ps, sb['c1wT'], xr_pad, N, bias_row=sb['c1b_row'][0:1, :])
    c1conv = work.tile([128, N], F32, tag="conv_out")
    for j0 in range(0, N, 512):
        nc.scalar.activation(out=c1conv[:, j0:j0 + 512],
                             in_=c1_ps[:, j0:j0 + 512], func=AF.Copy)
    dbg('c1conv', c1conv[:])
    c1T = ln_cpart(c1conv[:], sb['ln1_g'][:], sb['ln1_b'][:], N, relu=True,
                   tag="c1", out_dt=BF16, out_tag="ln_out_c1", msq_pool=ps_msq)
    dbg('c1T', c1T[:])

    # xs = raw reshape (N,C)->(C,N) of c1: 8 PE transposes of strided slices
    xs = cab_pool.tile([128, N], BF16, tag="cab_big")
    for nh in range(8):
        tpool = ps_sm if nh % 2 == 0 else ps_msq
        tp = tpool.tile([128, 128], BF16,
                        tag="psB" if nh % 2 == 0 else "psB2")
        src = bass.AP(tensor=c1T.tensor, offset=c1T.offset + nh,
                      ap=[[c1T.ap[0][0], 128], [8, 128]])
        nc.tensor.transpose(tp[:], src, sb['eye_b'][:])
        nc.scalar.activation(out=xs[:, nh * 128:(nh + 1) * 128], in_=tp[:],
                             func=AF.Copy)
    dbg('xs', xs[:])

    QT = cab_pool.tile([128, 8, 128], BF16)
    KT = cab_pool.tile([128, 8, 128], BF16)
    for i in range(8):
        tpool = ps_sm if i % 2 == 0 else ps_msq
        tp = tpool.tile([128, 256], F32,
                        tag="psB" if i % 2 == 0 else "psB2")
        mm(tp[:], xs[:, i * 128:(i + 1) * 128], sb['qkwT'][:],
           start=True, stop=False)
        mm(tp[:], sb['ones_row'][0:1, 0:128], sb['qkb_row'][:],
           start=False, stop=True)
        if i % 2 == 0:
            nc.vector.tensor_copy(out=QT[:, i, :], in_=tp[:, 0:128])
            nc.scalar.activation(out=KT[:, i, :], in_=tp[:, 128:256],
                                 func=AF.Copy)
        else:
            nc.scalar.activation(out=QT[:, i, :], in_=tp[:, 0:128],
                                 func=AF.Copy)
            nc.vector.tensor_copy(out=KT[:, i, :], in_=tp[:, 128:256])
    cc_ps = ps_sm.tile([128, 128], F32, tag="psB")
    for i in range(8):
        mm(cc_ps[:], QT[:, i, :], KT[:, i, :], start=(i == 0), stop=(i == 7))
    mx = work.tile([128, 1], F32, tag="sm_mx")
    nc.vector.tensor_reduce(out=mx[:], in_=cc_ps[:], axis=mybir.AxisListType.X,
                            op=AX.max)
    cc_sh = work.tile([128, 128], F32, tag="sm_sh")
    nc.vector.tensor_scalar(out=cc_sh[:], in0=cc_ps[:], scalar1=mx[:],
                            scalar2=None, op0=AX.subtract)
    cc_e = work.tile([128, 128], F32, tag="sm_e")
    nc.scalar.activation(out=cc_e[:], in_=cc_sh[:], func=AF.Exp)
    sm_s = work.tile([128, 1], F32, tag="sm_mx")
    nc.vector.tensor_reduce(out=sm_s[:], in_=cc_e[:], axis=mybir.AxisListType.X,
                            op=AX.add)
    sm_r = work.tile([128, 1], F32, tag="sm_r")
    nc.vector.reciprocal(out=sm_r[:], in_=sm_s[:])
    cc = cab_pool.tile([128, 128], BF16)
    nc.vector.tensor_scalar(out=cc[:], in0=cc_e[:], scalar1=sm_r[:],
                            scalar2=None, op0=AX.mult)
    dbg('cc', cc[:])

    xca = cab_pool.tile([128, N], F32, tag="cab_big")
    for j0 in range(0, N, 512):
        xca_ps = ps_sm.tile([128, 512], F32, tag="psB")
        mm(xca_ps[:], cc[:], c1T[:, j0:j0 + 512])
        nc.vector.scalar_tensor_tensor(out=xca[:, j0:j0 + 512],
                                       in0=c1T[:, j0:j0 + 512], scalar=2.0,
                                       in1=xca_ps[:], op0=AX.mult, op1=AX.add)
    xca_pad = cab_pool.tile([128, N + 2], F32R, tag="convpad")
    nc.gpsimd.tensor_copy(out=xca_pad[:, 0:1], in_=zeros4[:, 0:1])
    nc.gpsimd.tensor_copy(out=xca_pad[:, N + 1:N + 2], in_=zeros4[:, 0:1])
    nc.gpsimd.tensor_copy(out=xca_pad[:, 1:N + 1], in_=xca[:])
    c2_ps = ps_big.tile([128, N], F32, tag="psA")
    conv3(c2_ps, sb['c2wT'], xca_pad, N, bias_row=sb['c2b_row'][0:1, :])
    c2conv = work.tile([128, N], F32, tag="conv_out")
    for j0 in range(0, N, 512):
        nc.scalar.activation(out=c2conv[:, j0:j0 + 512],
                             in_=c2_ps[:, j0:j0 + 512], func=AF.Copy)
    c2T = ln_cpart(c2conv[:], sb['ln2_g'][:], sb['ln2_b'][:], N, relu=True,
                   tag="c2", out_tag="ln_out_seq", msq_pool=ps_msq)
    x2T = persist.tile([128, N], F32)
    for j0 in range(0, N, 512):
        nc.vector.tensor_tensor(out=x2T[:, j0:j0 + 512],
                                in0=sb['xT'][:, j0:j0 + 512],
                                in1=c2T[:, j0:j0 + 512], op=AX.add)
    dbg('x2T', x2T[:])
    cab_ctx.close()

    # =======================================================================
    # Phase 2: mamba front (duplicated within each pair)
    # =======================================================================
    xnT = ln_cpart(x2T[:], sb['mln_g'][:], sb['mln_b'][:], N, tag="mln",
                   out_dt=F32R, out_tag="ln_out_seq", msq_pool=ps_msq)
    msq_ctx.close()
    xi_t = [persist.tile([128, N], BF16, tag='xi0', name='xi0'),
            front_pool.tile([128, N], BF16, tag='xi1', name='xi1')]
    zT = front_pool.tile([128, N], F32)
    silu_z = persist.tile([128, N], F32)
    xpad_t = [front_pool.tile([128, N + 3], BF16, tag=f'xpad{i}', name=f'xpad{i}') for i in range(2)]
    for g in range(3):
        for j0 in range(0, N, 512):
            pj = ps_sm.tile([128, 512], F32, tag="psB")
            mm(pj[:], sb['in_wT'][:, g * 128:(g + 1) * 128], xnT[:, j0:j0 + 512])
            if (g + j0 // 512) % 2 == 0:
                dst_ap = (xpad_t[g][:, 3 + j0:3 + j0 + 512] if g < 2
                          else zT[:, j0:j0 + 512])
                nc.scalar.activation(out=dst_ap, in_=pj[:], func=AF.Copy)
            elif g < 2:
                nc.vector.tensor_copy(out=xpad_t[g][:, 3 + j0:3 + j0 + 512],
                                      in_=pj[:])
            else:
                nc.vector.tensor_copy(out=zT[:, j0:j0 + 512], in_=pj[:])
    nc.gpsimd.tensor_copy(out=xpad_t[0][:, 0:3], in_=zeros4[:, 0:3])
    nc.gpsimd.tensor_copy(out=xpad_t[1][:, 0:3], in_=zeros4[:, 0:3])
    for j0 in range(0, N, 512):
        sgz = work.tile([128, 512], F32, tag="sg", bufs=2, name="sg_z")
        nc.scalar.activation(out=sgz[:], in_=zT[:, j0:j0 + 512],
                             func=AF.Sigmoid)
        nc.gpsimd.tensor_tensor(out=silu_z[:, j0:j0 + 512],
                                in0=zT[:, j0:j0 + 512], in1=sgz[:],
                                op=AX.mult)
    for g in range(2):
        cps = ps_big.tile([128, N], F32, tag="psA")
        for j0 in range(0, N, 512):
            for k in range(DC):
                mm(cps[:, j0:j0 + 512], sb['conv_diag'][:, g * DC + k, :],
                   xpad_t[g][:, k + j0:k + j0 + 512],
                   start=(k == 0), stop=(k == 3))
            sgx = work.tile([128, 512], F32, tag="sg", bufs=2, name="sg_xi")
            nc.scalar.activation(out=sgx[:], in_=cps[:, j0:j0 + 512],
                                 func=AF.Sigmoid,
                                 bias=sb['convb'][:, g:g + 1])
            xc = work.tile([128, 512], F32, tag="xc_chunk", bufs=2,
                           name="xc")
            nc.vector.tensor_scalar(out=xc[:], in0=cps[:, j0:j0 + 512],
                                    scalar1=1.0,
                                    scalar2=sb['convb'][:, g:g + 1],
                                    op0=AX.mult, op1=AX.add)
            eng = nc.vector if g == 0 else nc.gpsimd
            eng.tensor_tensor(out=xi_t[g][:, j0:j0 + 512], in0=xc[:],
                              in1=sgx[:], op=AX.mult)
    dbg('xi0', xi_t[0][:])
    xi_r = xi_t
    dtl = front_pool.tile([8, N], F32R)
    BmT = persist.tile([S, N], BF16)
    CmT = persist.tile([S, N], BF16)
    cp_i = 0
    for (dst, wname, Msz) in ((dtl, 'xpw_dtl', R), (BmT, 'xpw_B', S),
                              (CmT, 'xpw_C', S)):
        for j0 in range(0, N, 512):
            pj = ps_sm.tile([Msz, 512], F32, tag="psB")
            for kk in range(2):
                mm(pj[:], sb[wname][:, kk, :], xi_r[kk][:, j0:j0 + 512],
                   start=(kk == 0), stop=(kk == 1))
            if cp_i % 2 == 0:
                nc.scalar.activation(out=dst[:, j0:j0 + 512], in_=pj[:],
                                     func=AF.Copy)
            else:
                nc.vector.tensor_copy(out=dst[:, j0:j0 + 512], in_=pj[:])
            cp_i += 1
    dbg('BmT', BmT[:]); dbg('CmT', CmT[:]); dbg('dtl', dtl[:])
    dtT = front_pool.tile([128, N], F32)
    for j0 in range(0, N, 512):
        pj = ps_sm.tile([128, 512], F32, tag="psB", name="pj_dt")
        mm(pj[:], sb['dtwT'][:], dtl[:, j0:j0 + 512])
        nc.scalar.activation(out=dtT[:, j0:j0 + 512], in_=pj[:],
                             func=AF.Exp, bias=sb['dtb_col'][:])
    for j0 in range(0, N, 512):
        nc.scalar.activation(out=dtT[:, j0:j0 + 512], in_=dtT[:, j0:j0 + 512],
                             func=AF.Ln, bias=one_col[:])
    dbg('dtT', dtT[:])
    dtb16 = persist.tile([128, N], BF16)
    for j0 in range(0, N, 512):
        nc.gpsimd.tensor_copy(out=dtb16[:, j0:j0 + 512],
                              in_=dtT[:, j0:j0 + 512])
    dtxT = front_pool.tile([128, N], BF16)
    for j0 in range(0, N, 512):
        nc.vector.tensor_tensor(out=dtxT[:, j0:j0 + 512],
                                in0=dtT[:, j0:j0 + 512],
                                in1=xi_t[0][:, j0:j0 + 512], op=AX.mult)
        nc.sync.dma_start(out=dtx_d[:, j0:j0 + 512], in_=dtxT[:, j0:j0 + 512])
    front_ctx.close()
    psy_ctx = contextlib.ExitStack()
    ps_y = psy_ctx.enter_context(tc.tile_pool(name="ps_y", bufs=1,
                                              space="PSUM"))
    scan_pool = ctx.enter_context(tc.tile_pool(name="scan", bufs=2))
    mkgu_pool = scan_pool
    # MKGU weight loads are chunked and interleaved into the scan loop below
    # so they ride SP's slack instead of stalling the post-scan phase.
    mkgu_loads = []
    for name in sorted(MKGU_INS):
        shape, dt = IN_SPECS[name]
        t = mkgu_pool.tile(list(shape), dt, tag=name, name="in_" + name,
                           bufs=1)
        sb[name] = t
        if len(shape) == 3 and shape[1] > 8:
            for k in range(0, shape[1], 4):
                k1 = min(k + 4, shape[1])
                mkgu_loads.append((t[:, k:k1, :], I[name][:, k:k1, :]))
        else:
            mkgu_loads.append((t[:], I[name]))

    # =======================================================================
    # Phase 3: selective scan over my 128 d's.
    # Per d: PE broadcasts the dt row into PSUM (ones[1,128] matmul from the
    # single-partition slice), Act exponentiates with the per-state A column,
    # a stride-0 DRAM DMA broadcasts the dtx row, and the b/scan/g elementwise
    # chain is split between DVE and Pool to balance engine load. The y
    # reduction over states stays on PE via the sliding one-hot.
    # =======================================================================
    y_ps = ps_y.tile([128, N], F32)
    for dd in range(128):
        dtx_bc = scan_pool.tile([128, N], BF16, tag="dtx_bc", bufs=5)
        src = bass.AP(tensor=dtx_d.tensor, offset=dd * N,
                      ap=[[0, 128], [1, N]])
        nc.sync.dma_start(out=dtx_bc[:], in_=src)
        dt_ps = ps_big.tile([128, N], F32, tag="psA", name="dtps")
        for j0 in range(0, N, 512):
            mm(dt_ps[:, j0:j0 + 512], sb['sel'][:, dd, :],
               dtb16[:, j0:j0 + 512])
        a_t = scan_pool.tile([128, N], BF16, tag="a", bufs=4)
        nc.scalar.activation(out=a_t[:], in_=dt_ps[:], func=AF.Exp,
                             scale=sb['ATneg'][:, dd:dd + 1])
        b_t = scan_pool.tile([128, N], BF16, tag="b", bufs=4)
        nc.gpsimd.tensor_tensor(out=b_t[:], in0=BmT[:], in1=dtx_bc[:],
                                op=AX.mult)
        h_t = scan_pool.tile([128, N], BF16, tag="h", bufs=4)
        nc.vector.tensor_tensor_scan(out=h_t[:], data0=a_t[:],
                                     data1=b_t[:], initial=0.0,
                                     op0=AX.mult, op1=AX.add)
        g_t = scan_pool.tile([128, N], BF16, tag="g", bufs=4)
        g_eng = nc.vector if dd % 5 < 2 else nc.gpsimd
        g_eng.tensor_tensor(out=g_t[:], in0=h_t[:], in1=CmT[:], op=AX.mult)
        for j0 in range(0, N, 512):
            mm(y_ps[:, j0:j0 + 512], sb['Pones'][:, 128 - dd:256 - dd],
               g_t[:, j0:j0 + 512], start=(dd == 0), stop=(dd == 127))
        if dd >= 8 and dd - 8 < len(mkgu_loads):
            ldst, lsrc = mkgu_loads[dd - 8]
            nc.sync.dma_start(out=ldst, in_=lsrc)

    yg = work.tile([128, N], F32, tag="mk_a", name="yg")
    ygate = persist.tile([128, N], F32R)
    for j0 in range(0, N, 512):
        nc.vector.scalar_tensor_tensor(out=yg[:, j0:j0 + 512],
                                       in0=xi_t[0][:, j0:j0 + 512],
                                       scalar=sb['Dcol'][:],
                                       in1=y_ps[:, j0:j0 + 512],
                                       op0=AX.mult, op1=AX.add)
        nc.gpsimd.tensor_tensor(out=ygate[:, j0:j0 + 512],
                                in0=yg[:, j0:j0 + 512],
                                in1=silu_z[:, j0:j0 + 512], op=AX.mult)
    dbg('yscan', yg[:])
    psy_ctx.close()
    ps_tail = ctx.enter_context(tc.tile_pool(name="ps_tail", bufs=2,
                                             space="PSUM"))
    op_ps = ps_big.tile([128, N], F32, tag="psA")
    for j0 in range(0, N, 512):
        mm(op_ps[:, j0:j0 + 512], sb['outwT'][:], ygate[:, j0:j0 + 512])
    rs_in = persist.tile([128, 2 * EXT], BF16)
    nc.vector.memset(rs_in[:, 0:EXTL], 0.0)
    nc.vector.memset(rs_in[:, 2 * EXT - EXTL:], 0.0)
    nc.vector.scalar_tensor_tensor(out=rs_in[:, EXTL:EXT],
                                   in0=x2T[:, 0:EXT - EXTL], scalar=0.5,
                                   in1=op_ps[:, 0:EXT - EXTL],
                                   op0=AX.mult, op1=AX.add)
    nc.vector.scalar_tensor_tensor(out=rs_in[:, EXT:2 * EXT - EXTL],
                                   in0=x2T[:, NH - EXTL:N], scalar=0.5,
                                   in1=op_ps[:, NH - EXTL:N],
                                   op0=AX.mult, op1=AX.add)
    nc.sync.dma_start(out=rs_in_d[0], in_=rs_in[:, 0:EXT])
    nc.scalar.dma_start(out=rs_in_d[1], in_=rs_in[:, EXT:])
    nc.gpsimd.collective_compute("ReduceScatter", AX.add, replica_groups=groups,
                                 ins=[rs_in_d], outs=[rs_out_d])
    warm_ps = ps_sm.tile([128, 512], F32, tag="psB", name="warm")
    for _w in range(56):
        mm(warm_ps[:], sb['Pones'][:, 0:128], rs_in[:, 0:512],
           start=(_w == 0), stop=(_w == 55))
    x3e = persist.tile([128, EXT], BF16)
    nc.sync.dma_start(out=x3e[:, 0:288], in_=rs_out_d[:, 0:288])
    nc.scalar.dma_start(out=x3e[:, 288:], in_=rs_out_d[:, 288:])
    dbg('x3e', x3e[:])

    # =======================================================================
    # Phase 4: MKGU on my region
    # =======================================================================
    knT = ln_cpart(x3e[:], sb['kln_g'][:], sb['kln_b'][:], EXT, tag="kln",
                   out_dt=F32R, msq_pool=ps_tail)
    x_dc = mkgu_pool.tile([128, EXT], F32)
    x_mc = mkgu_pool.tile([128, EXT], BF16)
    for g in (1, 0):
        hp = ps_big.tile([128, EXT], F32, tag="psA")
        for j0 in range(0, EXT, 512):
            j1 = min(j0 + 512, EXT)
            mm(hp[:, j0:j1], sb['kpwT'][:, g * 128:(g + 1) * 128], knT[:, j0:j1])
        sg = work.tile([128, EXT], F32, tag="sg_h")
        nc.scalar.activation(out=sg[:], in_=hp[:], func=AF.Sigmoid,
                             bias=sb['kpb'][:, g:g + 1])
        hs = work.tile([128, EXT], F32, tag="hpre")
        nc.vector.tensor_scalar(out=hs[:], in0=hp[:], scalar1=1.0,
                                scalar2=sb['kpb'][:, g:g + 1],
                                op0=AX.mult, op1=AX.add)
        hsw = work.tile([128, EXT], F32, tag="hsw")
        nc.vector.tensor_tensor(out=hsw[:], in0=hs[:], in1=sg[:], op=AX.mult)
        dst, eng = ((x_dc, nc.vector) if g == 0 else (x_mc, nc.gpsimd))
        eng.tensor_tensor(out=dst[:], in0=hsw[:], in1=sb['hmask'][:],
                          op=AX.mult)
    dbg('xmc', x_mc[:])
    mc_ps = ps_big.tile([128, MCW], F32, tag="psA")
    for j0 in range(0, MCW, 512):
        j1 = min(j0 + 512, MCW)
        for t in range(31):
            mm(mc_ps[:, j0:j1], sb['mc_comb'][:, t, :],
               x_mc[:, t + 1 + j0:t + 1 + j1], start=(t == 0), stop=(t == 30))
    mcT = mkgu_pool.tile([128, MCW], BF16)
    nc.vector.scalar_tensor_tensor(out=mcT[:], in0=mc_ps[:],
                                   scalar=sb['mcb_col'][:], in1=sb['mcmask'][:],
                                   op0=AX.add, op1=AX.mult)
    dbg('mc', mcT[:])
    dw_ps = ps_big.tile([128, NH], F32, tag="psA")
    for k in range(31):
        mm(dw_ps[:], sb['dw_diag'][:, k, :], mcT[:, k + 1:k + 1 + NH],
           start=(k == 0), stop=(k == 30))
    sg1 = work.tile([128, NH], F32, tag="sg_dw")
    nc.scalar.activation(out=sg1[:], in_=dw_ps[:], func=AF.Sigmoid,
                         scale=sb['bn1_s'][:], bias=sb['bn1_sb'][:])
    bn1 = work.tile([128, NH], F32, tag="mk_a")
    nc.vector.tensor_scalar(out=bn1[:], in0=dw_ps[:], scalar1=sb['bn1_s'][:],
                            scalar2=sb['bn1_sb'][:], op0=AX.mult, op1=AX.add)
    dw_silu = work.tile([128, NH], F32, tag="mk_b")
    nc.gpsimd.tensor_tensor(out=dw_silu[:], in0=bn1[:], in1=sg1[:], op=AX.mult)
    dwmc = work.tile([128, NH], F32, tag="mk_a")
    nc.gpsimd.tensor_tensor(out=dwmc[:], in0=dw_silu[:],
                            in1=mcT[:, 16:16 + NH], op=AX.add)
    warm2 = ps_tail.tile([128, 256], F32, tag="psB2", name="warm2")
    for _w in range(8):
        mm(warm2[:], sb['eye'][:], dwmc[:, 0:256],
           start=(_w == 0), stop=(_w == 7))
    sg2 = work.tile([128, NH], F32, tag="sg_dw")
    nc.scalar.activation(out=sg2[:], in_=dwmc[:], func=AF.Sigmoid,
                         scale=sb['bn2_s'][:], bias=sb['bn2_b'][:])
    bn2 = work.tile([128, NH], F32, tag="mk_b")
    nc.vector.tensor_scalar(out=bn2[:], in0=dwmc[:], scalar1=sb['bn2_s'][:],
                            scalar2=sb['bn2_b'][:], op0=AX.mult, op1=AX.add)
    bn2s = work.tile([128, NH], F32, tag="mk_c")
    nc.gpsimd.tensor_tensor(out=bn2s[:], in0=bn2[:], in1=sg2[:], op=AX.mult)
    outc = work.tile([128, NH], F32, tag="mk_b")
    nc.vector.tensor_tensor(out=outc[:], in0=bn2s[:],
                            in1=x_dc[:, EXTL:EXTL + NH], op=AX.mult)
    x4 = work.tile([128, NH], F32, tag="mk_a")
    nc.gpsimd.tensor_tensor(out=x4[:], in0=outc[:], in1=x3e[:, EXTL:EXTL + NH],
                            op=AX.add)
    x4n = ln_cpart(x4[:], sb['pn_g'][:], sb['pn_b'][:], NH, tag="pn",
                   msq_pool=ps_tail)
    for j in range(4):
        tp = ps_sm.tile([128, 128], F32, tag="psB")
        nc.tensor.transpose(tp[:], x4n[:, j * 128:(j + 1) * 128], sb['eye'][:])
        ot = work.tile([128, 128], F32, tag="out_sb", bufs=4)
        nc.vector.tensor_copy(out=ot[:], in_=tp[:])
        qeng = nc.sync if j % 2 == 0 else nc.scalar
        qeng.dma_start(out=out_half[j * 128:(j + 1) * 128, :], in_=ot[:])

    ctx.close()
    return nc


# --------------------------------------------------------------------------
# Entry point
# --------------------------------------------------------------------------
_CACHE = {}


def _get_nc():
    if "nc" not in _CACHE:
        nc = bacc.Bacc("TRN2", target_bir_lowering=False, debug=False,
                       num_devices=N_CORES)
        build(nc)
        nc.finalize()
        _CACHE["nc"] = nc
    return _CACHE["nc"]


def kernel(**inputs):
    import numpy as np
    nc = _get_nc()
    d = {k: np.asarray(v) for k, v in inputs.items()}
    in_maps = [prep_core_inputs(d, c // 2, c % 2) for c in range(N_CORES)]
    res = run_bass_kernel_spmd(nc, in_maps, core_ids=list(range(N_CORES)))
    return gather_output(res.results)



# revision 83
# speedup vs baseline: 1.0026x; 1.0019x over previous
"""Trainium2 Bass kernel for nn_CMKConMambaBlock (ConMamba block).

Sharding: 8 NeuronCores = 4 batch x 2 d_inner-halves. Single SPMD program;
per-core differences are injected purely through host-prepared inputs
(weights reordered so each core's d-half is local slice 0; the MKGU
N-region is routed by a pairwise ReduceScatter). The Mamba selective scan
runs as 128 per-d tensor_tensor_scan recurrences in an
[s=128 partitions, t=1024 free] layout. Per d, the dt row is broadcast on
the PE via a per-d one-hot selector matmul into PSUM, Act exponentiates it
with the per-state A column, the dtx row is broadcast by a stride-0 DRAM
DMA on the otherwise-idle SP queue, and the b/scan/g elementwise chain is
split across DVE and Pool (scan itself is DVE-only on HW; Pool never
touches PSUM). The y reduction over states stays on PE via the sliding
one-hot. MKGU weights stream in on SP slack chunked inside the scan loop;
conv biases ride the PE accumulation as rank-1 ones-row taps; the
ReduceScatter payload is bf16. Elementwise work is spread across
DVE/Pool/Act per the legacy cost model rates (DVE 2x for packed bf16,
Act/Pool dtype-agnostic).

kernel(**inputs) takes the full unsharded inputs (as produced by
setup_inputs()) and returns the full (4, 1024, 128) float32 output.
"""
import sys
for _p in ("/opt/trn_rl_repo", "/root/.axon_site/_ro/trn_rl_repo"):
    if _p not in sys.path:
        sys.path.append(_p)


import contextlib
import numpy as np
import ml_dtypes

import concourse.bass as bass
import concourse.bacc as bacc
from concourse.bass_utils import run_bass_kernel_spmd
import concourse.tile as tile
from concourse import mybir

F32 = mybir.dt.float32
F32R = mybir.dt.float32r
BF16 = mybir.dt.bfloat16
AX = mybir.AluOpType
AF = mybir.ActivationFunctionType

B, N, C = 4, 1024, 128
DI, S, R, DC = 256, 128, 8, 4
EPS = 1e-5
NH = 512          # N half
EXTL = 32         # halo for MKGU region
EXT = NH + 2 * EXTL   # 576
MCW = NH + 32     # mc width needed for dw conv: [n0-16, n0+528)
N_CORES = 8
BF = ml_dtypes.bfloat16


# --------------------------------------------------------------------------
# Host-side input prep (numpy only; layout + weight reorg).
# --------------------------------------------------------------------------
def prep_core_inputs(d, b, q):
    """d: dict of full inputs (numpy). Returns in_map for core 2*b+q."""
    f32 = np.float32

    def bf(x):
        return np.ascontiguousarray(np.asarray(x, f32).astype(BF))

    def cc(x):
        return np.ascontiguousarray(np.asarray(x, f32))

    m = {}
    x = np.asarray(d['x'], f32)
    m['xT'] = cc(x[b].T)                                     # [C,N]
    m['xr'] = bf(x[b].reshape(C, N))                         # raw reshape
    m['eye'] = np.eye(128, dtype=f32)
    m['eye_b'] = bf(m['eye'])
    Pn = np.zeros((128, 257), f32)
    Pn[:, 128] = 1.0
    m['Pones'] = bf(Pn)
    sel = np.zeros((128, 128, 128), f32)
    for dd in range(128):
        sel[dd, dd, :] = 1.0
    m['sel'] = bf(sel)
    m['Jc'] = np.full((128, 128), 1.0 / C, f32)
    m['Jc_b'] = bf(m['Jc'])
    m['Jc_r'] = m['Jc'].copy()
    m['ones_row'] = np.ones((1, 512), f32)

    # ---- CAB ----
    c1w = np.asarray(d['cab_c1_w'], f32)                     # [O,I,3]
    m['c1wT'] = bf(c1w.transpose(1, 2, 0))                   # [I=128,3,O=128]
    m['c1b_row'] = cc(np.asarray(d['cab_c1_b'], f32)[None, :])
    m['ln1_g'] = cc(np.asarray(d['cab_ln1_g'], f32)[:, None])
    m['ln1_b'] = cc(np.asarray(d['cab_ln1_b'], f32)[:, None])
    m['qkwT'] = bf(np.concatenate([np.asarray(d['ca_q_w'], f32).T,
                                   np.asarray(d['ca_k_w'], f32).T], 1))
    m['qkb_row'] = cc(np.concatenate([np.asarray(d['ca_q_b'], f32),
                                      np.asarray(d['ca_k_b'], f32)])[None, :])
    c2w = np.asarray(d['cab_c2_w'], f32)
    m['c2wT'] = cc(c2w.transpose(1, 2, 0))
    m['c2b_row'] = cc(np.asarray(d['cab_c2_b'], f32)[None, :])
    m['ln2_g'] = cc(np.asarray(d['cab_ln2_g'], f32)[:, None])
    m['ln2_b'] = cc(np.asarray(d['cab_ln2_b'], f32)[:, None])

    # ---- mamba front ----
    m['mln_g'] = cc(np.asarray(d['m_ln_g'], f32)[:, None])
    m['mln_b'] = cc(np.asarray(d['m_ln_b'], f32)[:, None])
    order = np.concatenate([np.arange(q * 128, (q + 1) * 128),
                            np.arange((1 - q) * 128, (2 - q) * 128)])
    in_w = np.asarray(d['m_in_w'], f32)                      # [512, C]
    w_xi = in_w[:DI][order]
    w_z = in_w[DI:][order[:128]]
    m['in_wT'] = cc(np.concatenate([w_xi, w_z], 0).T)        # [C,384]
    cw = np.asarray(d['m_conv_w'], f32)[order]               # [256, 4]
    diag = np.zeros((128, 2 * DC, 128), f32)
    for h in range(2):
        for k in range(DC):
            np.fill_diagonal(diag[:, h * DC + k, :], cw[h * 128:(h + 1) * 128, k])
    m['conv_diag'] = bf(diag)                                # [128, 8, 128]
    cb = np.asarray(d['m_conv_b'], f32)[order]
    m['convb'] = cc(cb.reshape(2, 128).T)                    # [128, 2]
    xp_w = np.asarray(d['m_xproj_w'], f32)                   # [264, 256]
    xp_wT = xp_w[:, order].T                                 # [256, 264]
    # pack k-halves into free dim: [128, 2, X]
    m['xpw_dtl'] = bf(xp_wT[:, :R].reshape(2, 128, R).transpose(1, 0, 2))
    m['xpw_B'] = bf(xp_wT[:, R:R + S].reshape(2, 128, S).transpose(1, 0, 2))
    m['xpw_C'] = bf(xp_wT[:, R + S:].reshape(2, 128, S).transpose(1, 0, 2))
    dt_w = np.asarray(d['m_dt_w'], f32)                      # [256, 8]
    m['dtwT'] = cc(dt_w[order[:128]].T)                      # [8, 128]
    m['dtb_col'] = cc(np.asarray(d['m_dt_b'], f32)[order[:128], None])
    A = -np.exp(np.asarray(d['m_Alog'], f32))                # [256, 128]
    m['ATneg'] = cc(A[order[:128]].T)                        # [S, 128]
    m['Dcol'] = cc(np.asarray(d['m_D'], f32)[order[:128], None])
    out_w = np.asarray(d['m_out_w'], f32)                    # [C, 256]
    m['outwT'] = cc(out_w[:, order[:128]].T)                 # [128, C]

    # ---- MKGU ----
    m['kln_g'] = cc(np.asarray(d['k_ln_g'], f32)[:, None])
    m['kln_b'] = cc(np.asarray(d['k_ln_b'], f32)[:, None])
    kp_w = np.asarray(d['k_proj_w'], f32)                    # [2C, C]
    m['kpwT'] = cc(kp_w.T)                                   # [C, 256]
    m['kpb'] = cc(np.asarray(d['k_proj_b'], f32).reshape(2, 128).T)
    mc_comb = np.zeros((128, 31, 128), f32)
    for ci, p in enumerate([3, 7, 11, 15]):
        w = np.asarray(d[f'k_c{ci + 1}_w'], f32)             # [32, 128, 2p+1]
        for k in range(2 * p + 1):
            sft = k - p
            mc_comb[:, sft + 15, 32 * ci:32 * (ci + 1)] = w[:, :, k].T
    m['mc_comb'] = bf(mc_comb)
    mcb = np.concatenate([np.asarray(d[f'k_c{i}_b'], f32) for i in range(1, 5)])
    m['mcb_col'] = cc(mcb[:, None])
    dww = np.asarray(d['k_dw_w'], f32)[:, 0, :]              # [128, 31]
    dwd = np.zeros((128, 31, 128), f32)
    for k in range(31):
        np.fill_diagonal(dwd[:, k, :], dww[:, k])
    m['dw_diag'] = bf(dwd)                                   # [128, 31, 128]
    s1 = np.asarray(d['k_bn1_g'], f32) / np.sqrt(np.float32(1.0 + EPS))
    m['bn1_s'] = cc(s1[:, None])
    # silu(bn1_s*(dw+dw_b)+bn1_b) folds the dw-conv bias into the BN bias
    m['bn1_sb'] = cc((s1 * np.asarray(d['k_dw_b'], f32)
                      + np.asarray(d['k_bn1_b'], f32))[:, None])
    s2 = np.asarray(d['k_bn2_g'], f32) / np.sqrt(np.float32(1.0 + EPS))
    m['bn2_s'] = cc(s2[:, None]); m['bn2_b'] = cc(np.asarray(d['k_bn2_b'], f32)[:, None])
    m['pn_g'] = cc(np.asarray(d['pn_g'], f32)[:, None])
    m['pn_b'] = cc(np.asarray(d['pn_b'], f32)[:, None])
    mask = np.ones((128, EXT), f32)
    if q == 0:
        mask[:, :EXTL] = 0.0
    else:
        mask[:, EXT - EXTL:] = 0.0
    m['hmask'] = mask
    mcm = np.ones((128, MCW), f32)
    if q == 0:
        mcm[:, :16] = 0.0
    else:
        mcm[:, MCW - 16:] = 0.0
    m['mcmask'] = mcm
    return m


def gather_output(results):
    out = np.zeros((B, N, C), np.float32)
    for b in range(B):
        out[b, :NH] = results[2 * b]['out_half']
        out[b, NH:] = results[2 * b + 1]['out_half']
    return out


IN_SPECS = {
    'xT': ([C, N], F32), 'xr': ([C, N], BF16), 'eye': ([128, 128], F32),
    'eye_b': ([128, 128], BF16),
    'Pones': ([128, 257], BF16), 'sel': ([128, 128, 128], BF16),
    'Jc': ([128, 128], F32), 'Jc_b': ([128, 128], BF16),
    'Jc_r': ([128, 128], F32R),
    'ones_row': ([1, 512], F32R),
    'c1wT': ([128, 3, 128], BF16), 'c1b_row': ([1, 128], F32R),
    'ln1_g': ([128, 1], F32), 'ln1_b': ([128, 1], F32),
    'qkwT': ([128, 256], BF16), 'qkb_row': ([1, 256], F32R),
    'c2wT': ([128, 3, 128], F32R), 'c2b_row': ([1, 128], F32R),
    'ln2_g': ([128, 1], F32), 'ln2_b': ([128, 1], F32),
    'mln_g': ([128, 1], F32), 'mln_b': ([128, 1], F32),
    'in_wT': ([C, 384], F32R),
    'conv_diag': ([128, 8, 128], BF16), 'convb': ([128, 2], F32),
    'xpw_dtl': ([128, 2, R], BF16), 'xpw_B': ([128, 2, S], BF16),
    'xpw_C': ([128, 2, S], BF16),
    'dtwT': ([R, 128], F32R), 'dtb_col': ([128, 1], F32),
    'ATneg': ([S, 128], F32), 'Dcol': ([128, 1], F32), 'outwT': ([128, C], F32R),
    'kln_g': ([128, 1], F32), 'kln_b': ([128, 1], F32),
    'kpwT': ([C, 2 * C], F32R), 'kpb': ([128, 2], F32),
    'mc_comb': ([128, 31, 128], BF16),
    'mcb_col': ([128, 1], F32),
    'dw_diag': ([128, 31, 128], BF16),
    'bn1_s': ([128, 1], F32), 'bn1_sb': ([128, 1], F32),
    'bn2_s': ([128, 1], F32), 'bn2_b': ([128, 1], F32),
    'pn_g': ([128, 1], F32), 'pn_b': ([128, 1], F32),
    'hmask': ([128, EXT], F32), 'mcmask': ([128, MCW], F32),
}


def build(nc, debug=(), y_split=3):
    """Emit the kernel IR. debug: iterable of intermediate names to DMA out.
    y_split: every y_split-th d runs its y-mul on gpsimd (0 = never)."""
    I = {}
    for name, (shape, dt) in IN_SPECS.items():
        I[name] = nc.dram_tensor(name, shape, dt, kind="ExternalInput").ap()
    out_half = nc.dram_tensor("out_half", [NH, C], F32, kind="ExternalOutput").ap()

    rs_in_d = nc.dram_tensor("rs_in_d", [2, C, EXT], BF16).ap()
    dtx_d = nc.dram_tensor("dtx_d", [128, N], BF16).ap()
    rs_out_d = nc.dram_tensor("rs_out_d", [C, EXT], BF16).ap()
    groups = [[0, 1], [2, 3], [4, 5], [6, 7]]

    ctx = contextlib.ExitStack()
    tc = ctx.enter_context(tile.TileContext(nc, num_cores=N_CORES))
    persist = ctx.enter_context(tc.tile_pool(name="persist", bufs=1))
    work = ctx.enter_context(tc.tile_pool(name="work", bufs=1))
    wln = ctx.enter_context(tc.tile_pool(name="wln", bufs=1))
    front_ctx = contextlib.ExitStack()
    front_pool = front_ctx.enter_context(tc.tile_pool(name="front", bufs=1))
    cab_ctx = contextlib.ExitStack()
    cab_pool = cab_ctx.enter_context(tc.tile_pool(name="cab", bufs=1))
    ps_big = ctx.enter_context(tc.tile_pool(name="ps_big", bufs=2, space="PSUM"))
    ps_sm = ctx.enter_context(tc.tile_pool(name="ps_sm", bufs=2, space="PSUM"))
    # front-LN-only second stats pool: lets mean/msq of consecutive LN chunks
    # double-buffer independently; closed before ps_y claims its space
    msq_ctx = contextlib.ExitStack()
    ps_msq = msq_ctx.enter_context(tc.tile_pool(name="ps_msq", bufs=2,
                                                space="PSUM"))


    def dbg(name, ap):
        if name in debug:
            t = nc.dram_tensor("dbg_" + name, list(ap.shape), ap.dtype,
                               kind="ExternalOutput").ap()
            nc.sync.dma_start(out=t, in_=ap)

    CAB_INS = {'xT', 'xr', 'qkwT', 'qkb_row', 'eye_b', 'c1wT', 'c2wT',
               'c1b_row', 'c2b_row', 'ln1_g', 'ln1_b', 'ln2_g', 'ln2_b'}
    FRONT_INS = {'in_wT', 'conv_diag', 'xpw_B', 'xpw_C', 'xpw_dtl', 'dtwT',
                 'dtb_col', 'convb', 'mln_g', 'mln_b'}
    MKGU_INS = {'mc_comb', 'dw_diag', 'hmask', 'kpwT', 'kpb', 'mcb_col',
                'bn1_s', 'bn1_sb', 'bn2_s', 'bn2_b', 'kln_g', 'mcmask',
                'kln_b', 'pn_g', 'pn_b'}
    sb = {}

    def load_input(name, pool):
        shape, dt = IN_SPECS[name]
        tg = "cab_big" if name == 'xr' else name
        t = pool.tile(list(shape), dt, tag=tg, name="in_" + name)
        nc.sync.dma_start(out=t[:], in_=I[name])
        sb[name] = t

    prio = ['xr', 'c1wT', 'c1b_row', 'ones_row', 'Jc', 'Jc_r', 'ln1_g',
            'ln1_b', 'eye_b', 'qkwT', 'qkb_row', 'eye', 'Pones']
    rest = [n for n in IN_SPECS
            if n not in MKGU_INS and n != 'sel' and n not in prio]
    for name in prio + rest:
        load_input(name, cab_pool if name in CAB_INS else (
            front_pool if name in FRONT_INS else persist))
    # sel lives in persist so its SBUF exists from the start and its load
    # overlaps the CAB phase (pool creation/release must nest LIFO).
    sel_t = persist.tile([128, 128, 128], BF16, tag="sel", name="in_sel")
    for ch in range(4):
        nc.sync.dma_start(out=sel_t[:, ch * 32:(ch + 1) * 32, :],
                          in_=I['sel'][:, ch * 32:(ch + 1) * 32, :])
    sb['sel'] = sel_t

    zeros4 = persist.tile([128, 4], F32, tag="zeros4")
    nc.vector.memset(zeros4[:], 0.0)
    eps_col = persist.tile([128, 1], F32, tag="eps_col")
    nc.vector.memset(eps_col[:], EPS)
    one_col = persist.tile([128, 1], F32, tag="one_col")
    nc.vector.memset(one_col[:], 1.0)

    def mm(out, lhsT, rhs, start=True, stop=True):
        nc.tensor.matmul(out, lhsT, rhs, start=start, stop=stop)

    # ---- helpers ----------------------------------------------------------
    def ln_cpart(xT, g_col, b_col, width, relu=False, tag="ln", out_dt=F32,
                 out_tag=None, cw=256, msq_pool=None):
        """LayerNorm over the partition dim of xT [128, width].
        Emitted in cw-col chunks with chunk-local tiles so the serial
        mean/var/rstd/apply chain pipelines across chunks."""
        CW = cw
        outt = wln.tile([128, width], out_dt, tag=out_tag or ("ln_out_" + tag))
        for j0 in range(0, width, CW):
            j1 = min(j0 + CW, width)
            w = j1 - j0
            c = (slice(None), slice(j0, j1))
            xsq = wln.tile([128, CW], F32R, tag="ln_sq", bufs=2, name="ln_sq")
            nc.gpsimd.tensor_tensor(out=xsq[:, :w], in0=xT[c], in1=xT[c],
                                    op=AX.mult)
            mean_ps = ps_sm.tile([128, CW], F32, tag="psB", name="ln_mps")
            msq_ps = (msq_pool or ps_sm).tile([128, CW], F32, tag="psB2"
                      if msq_pool else "psB", name="ln_qps")
            jc = sb['Jc_b'] if xT.dtype == BF16 else sb['Jc']
            mm(mean_ps[:, :w], jc[:], xT[c])
            mm(msq_ps[:, :w], sb['Jc_r'][:], xsq[:, :w])
            mean = wln.tile([128, CW], F32, tag="ln_mean", bufs=2, name="ln_mean")
            nc.vector.tensor_copy(out=mean[:, :w], in_=mean_ps[:, :w])
            m2 = wln.tile([128, CW], F32, tag="ln_m2", bufs=2, name="ln_m2")
            nc.gpsimd.tensor_tensor(out=m2[:, :w], in0=mean[:, :w],
                                    in1=mean[:, :w], op=AX.mult)
            var = wln.tile([128, CW], F32, tag="ln_var", bufs=2, name="ln_var")
            nc.vector.tensor_tensor(out=var[:, :w], in0=msq_ps[:, :w],
                                    in1=m2[:, :w], op=AX.subtract)
            std = wln.tile([128, CW], F32, tag="ln_std", bufs=2, name="ln_std")
            nc.scalar.activation(out=std[:, :w], in_=var[:, :w], func=AF.Sqrt,
                                 bias=eps_col[:])
            rstd = wln.tile([128, CW], F32, tag="ln_rstd", bufs=2, name="ln_rstd")
            nc.vector.reciprocal(out=rstd[:, :w], in_=std[:, :w])
            xm = wln.tile([128, CW], F32, tag="ln_xm", bufs=2, name="ln_xm")
            nc.gpsimd.tensor_tensor(out=xm[:, :w], in0=xT[c], in1=mean[:, :w],
                                    op=AX.subtract)
            xn = wln.tile([128, CW], F32, tag="ln_xn", bufs=2, name="ln_xn")
            nc.vector.tensor_tensor(out=xn[:, :w], in0=xm[:, :w],
                                    in1=rstd[:, :w], op=AX.mult)
            if relu:
                nc.scalar.activation(out=outt[c], in_=xn[:, :w], func=AF.Relu,
                                     scale=g_col, bias=b_col)
            else:
                nc.vector.tensor_scalar(out=outt[c], in0=xn[:, :w],
                                        scalar1=g_col, scalar2=b_col,
                                        op0=AX.mult, op1=AX.add)
        return outt

    def silu_into(out_ap, in_ap, width, tag, eng=None):
        sg = work.tile([128, width], F32, tag="sg", name="sg_" + tag)
        nc.scalar.activation(out=sg[:], in_=in_ap, func=AF.Sigmoid)
        (eng or nc.vector).tensor_tensor(out=out_ap, in0=in_ap, in1=sg[:],
                                         op=AX.mult)

    def conv3(out_ps, wT3, xpad, width, bias_row=None):
        """out_ps[:,:width] = sum_k wT3[:,k,:].T @ xpad[:, k:k+width] (+bias)"""
        for j0 in range(0, width, 512):
            j1 = min(j0 + 512, width)
            for k in range(3):
                mm(out_ps[:, j0:j1], wT3[:, k, :], xpad[:, k + j0:k + j1],
                   start=(k == 0), stop=False)
            mm(out_ps[:, j0:j1], bias_row,
               sb['ones_row'][0:1, 0:j1 - j0], start=False, stop=True)

    # =======================================================================
    # Phase 1: CAB (duplicated within each pair)
    # =======================================================================
    xr_pad = cab_pool.tile([128, N + 2], BF16, tag="convpad_b")
    nc.gpsimd.tensor_copy(out=xr_pad[:, 0:1], in_=zeros4[:, 0:1])
    nc.gpsimd.tensor_copy(out=xr_pad[:, N + 1:N + 2], in_=zeros4[:, 0:1])
    nc.gpsimd.tensor_copy(out=xr_pad[:, 1:N + 1], in_=sb['xr'][:])
    c1_ps = ps_big.tile([128, N], F32, tag="psA")
    conv3(c1_ps, sb['c1wT'], xr_pad, N, bias_row=sb['c1b_row'][0:1, :])
    c1conv = work.tile([128, N], F32, tag="conv_out")
    for j0 in range(0, N, 512):
        nc.scalar.activation(out=c1conv[:, j0:j0 + 512],
                             in_=c1_ps[:, j0:j0 + 512], func=AF.Copy)
    dbg('c1conv', c1conv[:])
    c1T = ln_cpart(c1conv[:], sb['ln1_g'][:], sb['ln1_b'][:], N, relu=True,
                   tag="c1", out_dt=BF16, out_tag="ln_out_c1", msq_pool=ps_msq)
    dbg('c1T', c1T[:])

    # xs = raw reshape (N,C)->(C,N) of c1: 8 PE transposes of strided slices
    xs = cab_pool.tile([128, N], BF16, tag="cab_big")
    for nh in range(8):
        tpool = ps_sm if nh % 2 == 0 else ps_msq
        tp = tpool.tile([128, 128], BF16,
                        tag="psB" if nh % 2 == 0 else "psB2")
        src = bass.AP(tensor=c1T.tensor, offset=c1T.offset + nh,
                      ap=[[c1T.ap[0][0], 128], [8, 128]])
        nc.tensor.transpose(tp[:], src, sb['eye_b'][:])
        nc.scalar.activation(out=xs[:, nh * 128:(nh + 1) * 128], in_=tp[:],
                             func=AF.Copy)
    dbg('xs', xs[:])

    QT = cab_pool.tile([128, 8, 128], BF16)
    KT = cab_pool.tile([128, 8, 128], BF16)
    for i in range(8):
        tpool = ps_sm if i % 2 == 0 else ps_msq
        tp = tpool.tile([128, 256], F32,
                        tag="psB" if i % 2 == 0 else "psB2")
        mm(tp[:], xs[:, i * 128:(i + 1) * 128], sb['qkwT'][:],
           start=True, stop=False)
        mm(tp[:], sb['ones_row'][0:1, 0:128], sb['qkb_row'][:],
           start=False, stop=True)
        if i % 2 == 0:
            nc.vector.tensor_copy(out=QT[:, i, :], in_=tp[:, 0:128])
            nc.scalar.activation(out=KT[:, i, :], in_=tp[:, 128:256],
                                 func=AF.Copy)
        else:
            nc.scalar.activation(out=QT[:, i, :], in_=tp[:, 0:128],
                                 func=AF.Copy)
            nc.vector.tensor_copy(out=KT[:, i, :], in_=tp[:, 128:256])
    cc_ps = ps_sm.tile([128, 128], F32, tag="psB")
    for i in range(8):
        mm(cc_ps[:], QT[:, i, :], KT[:, i, :], start=(i == 0), stop=(i == 7))
    mx = work.tile([128, 1], F32, tag="sm_mx")
    nc.vector.tensor_reduce(out=mx[:], in_=cc_ps[:], axis=mybir.AxisListType.X,
                            op=AX.max)
    cc_sh = work.tile([128, 128], F32, tag="sm_sh")
    nc.vector.tensor_scalar(out=cc_sh[:], in0=cc_ps[:], scalar1=mx[:],
                            scalar2=None, op0=AX.subtract)
    cc_e = work.tile([128, 128], F32, tag="sm_e")
    nc.scalar.activation(out=cc_e[:], in_=cc_sh[:], func=AF.Exp)
    sm_s = work.tile([128, 1], F32, tag="sm_mx")
    nc.vector.tensor_reduce(out=sm_s[:], in_=cc_e[:], axis=mybir.AxisListType.X,
                            op=AX.add)
    sm_r = work.tile([128, 1], F32, tag="sm_r")
    nc.vector.reciprocal(out=sm_r[:], in_=sm_s[:])
    cc = cab_pool.tile([128, 128], BF16)
    nc.vector.tensor_scalar(out=cc[:], in0=cc_e[:], scalar1=sm_r[:],
                            scalar2=None, op0=AX.mult)
    dbg('cc', cc[:])

    xca = cab_pool.tile([128, N], F32, tag="cab_big")
    for j0 in range(0, N, 512):
        xca_ps = ps_sm.tile([128, 512], F32, tag="psB")
        mm(xca_ps[:], cc[:], c1T[:, j0:j0 + 512])
        nc.vector.scalar_tensor_tensor(out=xca[:, j0:j0 + 512],
                                       in0=c1T[:, j0:j0 + 512], scalar=2.0,
                                       in1=xca_ps[:], op0=AX.mult, op1=AX.add)
    xca_pad = cab_pool.tile([128, N + 2], F32R, tag="convpad")
    nc.gpsimd.tensor_copy(out=xca_pad[:, 0:1], in_=zeros4[:, 0:1])
    nc.gpsimd.tensor_copy(out=xca_pad[:, N + 1:N + 2], in_=zeros4[:, 0:1])
    nc.gpsimd.tensor_copy(out=xca_pad[:, 1:N + 1], in_=xca[:])
    c2_ps = ps_big.tile([128, N], F32, tag="psA")
    conv3(c2_ps, sb['c2wT'], xca_pad, N, bias_row=sb['c2b_row'][0:1, :])
    c2conv = work.tile([128, N], F32, tag="conv_out")
    for j0 in range(0, N, 512):
        nc.scalar.activation(out=c2conv[:, j0:j0 + 512],
                             in_=c2_ps[:, j0:j0 + 512], func=AF.Copy)
    c2T = ln_cpart(c2conv[:], sb['ln2_g'][:], sb['ln2_b'][:], N, relu=True,
                   tag="c2", out_tag="ln_out_seq", msq_pool=ps_msq)
    x2T = persist.tile([128, N], F32)
    for j0 in range(0, N, 512):
        nc.vector.tensor_tensor(out=x2T[:, j0:j0 + 512],
                                in0=sb['xT'][:, j0:j0 + 512],
                                in1=c2T[:, j0:j0 + 512], op=AX.add)
    dbg('x2T', x2T[:])
    cab_ctx.close()

    # =======================================================================
    # Phase 2: mamba front (duplicated within each pair)
    # =======================================================================
    xnT = ln_cpart(x2T[:], sb['mln_g'][:], sb['mln_b'][:], N, tag="mln",
                   out_dt=F32R, out_tag="ln_out_seq", msq_pool=ps_msq)
    msq_ctx.close()
    xi_t = [persist.tile([128, N], BF16, tag='xi0', name='xi0'),
            front_pool.tile([128, N], BF16, tag='xi1', name='xi1')]
    zT = front_pool.tile([128, N], F32)
    silu_z = persist.tile([128, N], F32)
    xpad_t = [front_pool.tile([128, N + 3], BF16, tag=f'xpad{i}', name=f'xpad{i}') for i in range(2)]
    for g in range(3):
        for j0 in range(0, N, 512):
            pj = ps_sm.tile([128, 512], F32, tag="psB")
            mm(pj[:], sb['in_wT'][:, g * 128:(g + 1) * 128], xnT[:, j0:j0 + 512])
            if (g + j0 // 512) % 2 == 0:
                dst_ap = (xpad_t[g][:, 3 + j0:3 + j0 + 512] if g < 2
                          else zT[:, j0:j0 + 512])
                nc.scalar.activation(out=dst_ap, in_=pj[:], func=AF.Copy)
            elif g < 2:
                nc.vector.tensor_copy(out=xpad_t[g][:, 3 + j0:3 + j0 + 512],
                                      in_=pj[:])
            else:
                nc.vector.tensor_copy(out=zT[:, j0:j0 + 512], in_=pj[:])
    nc.gpsimd.tensor_copy(out=xpad_t[0][:, 0:3], in_=zeros4[:, 0:3])
    nc.gpsimd.tensor_copy(out=xpad_t[1][:, 0:3], in_=zeros4[:, 0:3])
    for j0 in range(0, N, 512):
        sgz = work.tile([128, 512], F32, tag="sg", bufs=2, name="sg_z")
        nc.scalar.activation(out=sgz[:], in_=zT[:, j0:j0 + 512],
                             func=AF.Sigmoid)
        nc.gpsimd.tensor_tensor(out=silu_z[:, j0:j0 + 512],
                                in0=zT[:, j0:j0 + 512], in1=sgz[:],
                                op=AX.mult)
    for g in range(2):
        cps = ps_big.tile([128, N], F32, tag="psA")
        for j0 in range(0, N, 512):
            for k in range(DC):
                mm(cps[:, j0:j0 + 512], sb['conv_diag'][:, g * DC + k, :],
                   xpad_t[g][:, k + j0:k + j0 + 512],
                   start=(k == 0), stop=(k == 3))
            sgx = work.tile([128, 512], F32, tag="sg", bufs=2, name="sg_xi")
            nc.scalar.activation(out=sgx[:], in_=cps[:, j0:j0 + 512],
                                 func=AF.Sigmoid,
                                 bias=sb['convb'][:, g:g + 1])
            xc = work.tile([128, 512], F32, tag="xc_chunk", bufs=2,
                           name="xc")
            nc.vector.tensor_scalar(out=xc[:], in0=cps[:, j0:j0 + 512],
                                    scalar1=1.0,
                                    scalar2=sb['convb'][:, g:g + 1],
                                    op0=AX.mult, op1=AX.add)
            eng = nc.vector if g == 0 else nc.gpsimd
            eng.tensor_tensor(out=xi_t[g][:, j0:j0 + 512], in0=xc[:],
                              in1=sgx[:], op=AX.mult)
    dbg('xi0', xi_t[0][:])
    xi_r = xi_t
    dtl = front_pool.tile([8, N], F32R)
    BmT = persist.tile([S, N], BF16)
    CmT = persist.tile([S, N], BF16)
    cp_i = 0
    for (dst, wname, Msz) in ((dtl, 'xpw_dtl', R), (BmT, 'xpw_B', S),
                              (CmT, 'xpw_C', S)):
        for j0 in range(0, N, 512):
            pj = ps_sm.tile([Msz, 512], F32, tag="psB")
            for kk in range(2):
                mm(pj[:], sb[wname][:, kk, :], xi_r[kk][:, j0:j0 + 512],
                   start=(kk == 0), stop=(kk == 1))
            if cp_i % 2 == 0:
                nc.scalar.activation(out=dst[:, j0:j0 + 512], in_=pj[:],
                                     func=AF.Copy)
            else:
                nc.vector.tensor_copy(out=dst[:, j0:j0 + 512], in_=pj[:])
            cp_i += 1
    dbg('BmT', BmT[:]); dbg('CmT', CmT[:]); dbg('dtl', dtl[:])
    dtT = front_pool.tile([128, N], F32)
    for j0 in range(0, N, 512):
        pj = ps_sm.tile([128, 512], F32, tag="psB", name="pj_dt")
        mm(pj[:], sb['dtwT'][:], dtl[:, j0:j0 + 512])
        nc.scalar.activation(out=dtT[:, j0:j0 + 512], in_=pj[:],
                             func=AF.Exp, bias=sb['dtb_col'][:])
    for j0 in range(0, N, 512):
        nc.scalar.activation(out=dtT[:, j0:j0 + 512], in_=dtT[:, j0:j0 + 512],
                             func=AF.Ln, bias=one_col[:])
    dbg('dtT', dtT[:])
    dtb16 = persist.tile([128, N], BF16)
    for j0 in range(0, N, 512):
        nc.gpsimd.tensor_copy(out=dtb16[:, j0:j0 + 512],
                              in_=dtT[:, j0:j0 + 512])
    dtxT = front_pool.tile([128, N], BF16)
    for j0 in range(0, N, 512):
        nc.vector.tensor_tensor(out=dtxT[:, j0:j0 + 512],
                                in0=dtT[:, j0:j0 + 512],
                                in1=xi_t[0][:, j0:j0 + 512], op=AX.mult)
        nc.sync.dma_start(out=dtx_d[:, j0:j0 + 512], in_=dtxT[:, j0:j0 + 512])
    front_ctx.close()
    psy_ctx = contextlib.ExitStack()
    ps_y = psy_ctx.enter_context(tc.tile_pool(name="ps_y", bufs=1,
                                              space="PSUM"))
    scan_pool = ctx.enter_context(tc.tile_pool(name="scan", bufs=2))
    mkgu_pool = scan_pool
    # MKGU weight loads are chunked and interleaved into the scan loop below
    # so they ride SP's slack instead of stalling the post-scan phase.
    mkgu_loads = []
    for name in sorted(MKGU_INS):
        shape, dt = IN_SPECS[name]
        t = mkgu_pool.tile(list(shape), dt, tag=name, name="in_" + name,
                           bufs=1)
        sb[name] = t
        if len(shape) == 3 and shape[1] > 8:
            for k in range(0, shape[1], 4):
                k1 = min(k + 4, shape[1])
                mkgu_loads.append((t[:, k:k1, :], I[name][:, k:k1, :]))
        else:
            mkgu_loads.append((t[:], I[name]))

    # =======================================================================
    # Phase 3: selective scan over my 128 d's.
    # Per d: PE broadcasts the dt row into PSUM (ones[1,128] matmul from the
    # single-partition slice), Act exponentiates with the per-state A column,
    # a stride-0 DRAM DMA broadcasts the dtx row, and the b/scan/g elementwise
    # chain is split between DVE and Pool to balance engine load. The y
    # reduction over states stays on PE via the sliding one-hot.
    # =======================================================================
    y_ps = ps_y.tile([128, N], F32)
    for dd in range(128):
        dtx_bc = scan_pool.tile([128, N], BF16, tag="dtx_bc", bufs=5)
        src = bass.AP(tensor=dtx_d.tensor, offset=dd * N,
                      ap=[[0, 128], [1, N]])
        nc.sync.dma_start(out=dtx_bc[:], in_=src)
        dt_ps = ps_big.tile([128, N], F32, tag="psA", name="dtps")
        for j0 in range(0, N, 512):
            mm(dt_ps[:, j0:j0 + 512], sb['sel'][:, dd, :],
               dtb16[:, j0:j0 + 512])
        a_t = scan_pool.tile([128, N], BF16, tag="a", bufs=4)
        nc.scalar.activation(out=a_t[:], in_=dt_ps[:], func=AF.Exp,
                             scale=sb['ATneg'][:, dd:dd + 1])
        b_t = scan_pool.tile([128, N], BF16, tag="b", bufs=4)
        nc.gpsimd.tensor_tensor(out=b_t[:], in0=BmT[:], in1=dtx_bc[:],
                                op=AX.mult)
        h_t = scan_pool.tile([128, N], BF16, tag="h", bufs=4)
        nc.vector.tensor_tensor_scan(out=h_t[:], data0=a_t[:],
                                     data1=b_t[:], initial=0.0,
                                     op0=AX.mult, op1=AX.add)
        g_t = scan_pool.tile([128, N], BF16, tag="g", bufs=4)
        g_eng = nc.vector if dd * 2 % 5 < 2 else nc.gpsimd
        g_eng.tensor_tensor(out=g_t[:], in0=h_t[:], in1=CmT[:], op=AX.mult)
        for j0 in range(0, N, 512):
            mm(y_ps[:, j0:j0 + 512], sb['Pones'][:, 128 - dd:256 - dd],
               g_t[:, j0:j0 + 512], start=(dd == 0), stop=(dd == 127))
        if dd >= 8 and dd - 8 < len(mkgu_loads):
            ldst, lsrc = mkgu_loads[dd - 8]
            nc.sync.dma_start(out=ldst, in_=lsrc)

    yg = work.tile([128, N], F32, tag="mk_a", name="yg")
    ygate = persist.tile([128, N], F32R)
    for j0 in range(0, N, 512):
        nc.vector.scalar_tensor_tensor(out=yg[:, j0:j0 + 512],
                                       in0=xi_t[0][:, j0:j0 + 512],
                                       scalar=sb['Dcol'][:],
                                       in1=y_ps[:, j0:j0 + 512],
                                       op0=AX.mult, op1=AX.add)
        nc.gpsimd.tensor_tensor(out=ygate[:, j0:j0 + 512],
                                in0=yg[:, j0:j0 + 512],
                                in1=silu_z[:, j0:j0 + 512], op=AX.mult)
    dbg('yscan', yg[:])
    psy_ctx.close()
    ps_tail = ctx.enter_context(tc.tile_pool(name="ps_tail", bufs=2,
                                             space="PSUM"))
    op_ps = ps_big.tile([128, N], F32, tag="psA")
    for j0 in range(0, N, 512):
        mm(op_ps[:, j0:j0 + 512], sb['outwT'][:], ygate[:, j0:j0 + 512])
    rs_in = persist.tile([128, 2 * EXT], BF16)
    nc.vector.memset(rs_in[:, 0:EXTL], 0.0)
    nc.vector.memset(rs_in[:, 2 * EXT - EXTL:], 0.0)
    nc.vector.scalar_tensor_tensor(out=rs_in[:, EXTL:EXT],
                                   in0=x2T[:, 0:EXT - EXTL], scalar=0.5,
                                   in1=op_ps[:, 0:EXT - EXTL],
                                   op0=AX.mult, op1=AX.add)
    nc.vector.scalar_tensor_tensor(out=rs_in[:, EXT:2 * EXT - EXTL],
                                   in0=x2T[:, NH - EXTL:N], scalar=0.5,
                                   in1=op_ps[:, NH - EXTL:N],
                                   op0=AX.mult, op1=AX.add)
    nc.sync.dma_start(out=rs_in_d[0], in_=rs_in[:, 0:EXT])
    nc.scalar.dma_start(out=rs_in_d[1], in_=rs_in[:, EXT:])
    nc.gpsimd.collective_compute("ReduceScatter", AX.add, replica_groups=groups,
                                 ins=[rs_in_d], outs=[rs_out_d])
    warm_ps = ps_sm.tile([128, 512], F32, tag="psB", name="warm")
    for _w in range(56):
        mm(warm_ps[:], sb['Pones'][:, 0:128], rs_in[:, 0:512],
           start=(_w == 0), stop=(_w == 55))
    x3e = persist.tile([128, EXT], BF16)
    nc.sync.dma_start(out=x3e[:, 0:288], in_=rs_out_d[:, 0:288])
    nc.scalar.dma_start(out=x3e[:, 288:], in_=rs_out_d[:, 288:])
    dbg('x3e', x3e[:])

    # =======================================================================
    # Phase 4: MKGU on my region
    # =======================================================================
    knT = ln_cpart(x3e[:], sb['kln_g'][:], sb['kln_b'][:], EXT, tag="kln",
                   out_dt=F32R, msq_pool=ps_tail)
    x_dc = mkgu_pool.tile([128, EXT], F32)
    x_mc = mkgu_pool.tile([128, EXT], BF16)
    for g in (1, 0):
        hp = ps_big.tile([128, EXT], F32, tag="psA")
        for j0 in range(0, EXT, 512):
            j1 = min(j0 + 512, EXT)
            mm(hp[:, j0:j1], sb['kpwT'][:, g * 128:(g + 1) * 128], knT[:, j0:j1])
        sg = work.tile([128, EXT], F32, tag="sg_h")
        nc.scalar.activation(out=sg[:], in_=hp[:], func=AF.Sigmoid,
                             bias=sb['kpb'][:, g:g + 1])
        hs = work.tile([128, EXT], F32, tag="hpre")
        nc.vector.tensor_scalar(out=hs[:], in0=hp[:], scalar1=1.0,
                                scalar2=sb['kpb'][:, g:g + 1],
                                op0=AX.mult, op1=AX.add)
        hsw = work.tile([128, EXT], F32, tag="hsw")
        nc.vector.tensor_tensor(out=hsw[:], in0=hs[:], in1=sg[:], op=AX.mult)
        dst, eng = ((x_dc, nc.vector) if g == 0 else (x_mc, nc.gpsimd))
        eng.tensor_tensor(out=dst[:], in0=hsw[:], in1=sb['hmask'][:],
                          op=AX.mult)
    dbg('xmc', x_mc[:])
    mc_ps = ps_big.tile([128, MCW], F32, tag="psA")
    for j0 in range(0, MCW, 512):
        j1 = min(j0 + 512, MCW)
        for t in range(31):
            mm(mc_ps[:, j0:j1], sb['mc_comb'][:, t, :],
               x_mc[:, t + 1 + j0:t + 1 + j1], start=(t == 0), stop=(t == 30))
    mcT = mkgu_pool.tile([128, MCW], BF16)
    nc.vector.scalar_tensor_tensor(out=mcT[:], in0=mc_ps[:],
                                   scalar=sb['mcb_col'][:], in1=sb['mcmask'][:],
                                   op0=AX.add, op1=AX.mult)
    dbg('mc', mcT[:])
    dw_ps = ps_big.tile([128, NH], F32, tag="psA")
    for k in range(31):
        mm(dw_ps[:], sb['dw_diag'][:, k, :], mcT[:, k + 1:k + 1 + NH],
           start=(k == 0), stop=(k == 30))
    sg1 = work.tile([128, NH], F32, tag="sg_dw")
    nc.scalar.activation(out=sg1[:], in_=dw_ps[:], func=AF.Sigmoid,
                         scale=sb['bn1_s'][:], bias=sb['bn1_sb'][:])
    bn1 = work.tile([128, NH], F32, tag="mk_a")
    nc.vector.tensor_scalar(out=bn1[:], in0=dw_ps[:], scalar1=sb['bn1_s'][:],
                            scalar2=sb['bn1_sb'][:], op0=AX.mult, op1=AX.add)
    dw_silu = work.tile([128, NH], F32, tag="mk_b")
    nc.gpsimd.tensor_tensor(out=dw_silu[:], in0=bn1[:], in1=sg1[:], op=AX.mult)
    dwmc = work.tile([128, NH], F32, tag="mk_a")
    nc.gpsimd.tensor_tensor(out=dwmc[:], in0=dw_silu[:],
                            in1=mcT[:, 16:16 + NH], op=AX.add)
    warm2 = ps_tail.tile([128, 256], F32, tag="psB2", name="warm2")
    for _w in range(8):
        mm(warm2[:], sb['eye'][:], dwmc[:, 0:256],
           start=(_w == 0), stop=(_w == 7))
    sg2 = work.tile([128, NH], F32, tag="sg_dw")
    nc.scalar.activation(out=sg2[:], in_=dwmc[:], func=AF.Sigmoid,
                         scale=sb['bn2_s'][:], bias=sb['bn2_b'][:])
    bn2 = work.tile([128, NH], F32, tag="mk_b")
    nc.vector.tensor_scalar(out=bn2[:], in0=dwmc[:], scalar1=sb['bn2_s'][:],
                            scalar2=sb['bn2_b'][:], op0=AX.mult, op1=AX.add)
    bn2s = work.tile([128, NH], F32, tag="mk_c")
    nc.gpsimd.tensor_tensor(out=bn2s[:], in0=bn2[:], in1=sg2[:], op=AX.mult)
    outc = work.tile([128, NH], F32, tag="mk_b")
    nc.vector.tensor_tensor(out=outc[:], in0=bn2s[:],
                            in1=x_dc[:, EXTL:EXTL + NH], op=AX.mult)
    x4 = work.tile([128, NH], F32, tag="mk_a")
    nc.gpsimd.tensor_tensor(out=x4[:], in0=outc[:], in1=x3e[:, EXTL:EXTL + NH],
                            op=AX.add)
    x4n = ln_cpart(x4[:], sb['pn_g'][:], sb['pn_b'][:], NH, tag="pn",
                   msq_pool=ps_tail)
    for j in range(4):
        tp = ps_sm.tile([128, 128], F32, tag="psB")
        nc.tensor.transpose(tp[:], x4n[:, j * 128:(j + 1) * 128], sb['eye'][:])
        ot = work.tile([128, 128], F32, tag="out_sb", bufs=4)
        nc.vector.tensor_copy(out=ot[:], in_=tp[:])
        qeng = nc.sync if j % 2 == 0 else nc.scalar
        qeng.dma_start(out=out_half[j * 128:(j + 1) * 128, :], in_=ot[:])

    ctx.close()
    return nc


# --------------------------------------------------------------------------
# Entry point
# --------------------------------------------------------------------------
_CACHE = {}


def _get_nc():
    if "nc" not in _CACHE:
        nc = bacc.Bacc("TRN2", target_bir_lowering=False, debug=False,
                       num_devices=N_CORES)
        build(nc)
        nc.finalize()
        _CACHE["nc"] = nc
    return _CACHE["nc"]


def kernel(**inputs):
    import numpy as np
    nc = _get_nc()
    d = {k: np.asarray(v) for k, v in inputs.items()}
    in_maps = [prep_core_inputs(d, c // 2, c % 2) for c in range(N_CORES)]
    res = run_bass_kernel_spmd(nc, in_maps, core_ids=list(range(N_CORES)))
    return gather_output(res.results)

